# revision 1
# baseline (speedup 1.0000x reference)
"""Windowed local self-attention (CrossAttention module with the context-
overwrite bug faithfully reproduced) on 8 Trainium2 NeuronCores.

Full-input contract: kernel(**inputs) takes the unsharded tensors and
returns the full (4, 4096, 1024) output. Internally the 64 independent
windows of 256 tokens are data-parallel sharded 8-per-core; the four
projection weights are broadcast to every core. No collectives needed.

Per-core pipeline (window = 256 tokens, H=16 heads, DH=64):
  X  --PE transpose-->  XT [d, i]
  qT = Wq.T @ X.T   (lhsT=Wq tiles,  rhs=XT)          [o, i]
  kT = Wk.T @ X.T                                      [o, i]
  v  = X @ Wv       (lhsT=XT tiles,  rhs=Wv)           [j, o]
  per head h:
    simT = kT_h.T-free @ qT_h   -> [j, i] in PSUM     (j on partitions)
    es   = exp(0.125 * simT)    (ACT, PSUM->SBUF)
    S    = ones[j,64].T @ es    -> [64, i] broadcast row-sums (PE)
    rS   = 1/S                  (DVE reciprocal)
    o2u  = v_h.T-free @ es      -> [d, i] in PSUM      (AV matmul)
    o2T  = o2u * rS             (DVE, writes stacked [o, i] SBUF)
  Y = o2T.T @ Wo       (lhsT=o2T tiles, rhs=Wo; zero bias added host-side)
All matmul operands are bitcast to float32r: full fp32 bits, 1 cycle/row
on the PE at moving free-dim >= 256 (vs 4 cycles/row for plain float32).
"""

import numpy as np

import concourse.bass as bass
import concourse.mybir as mybir
import concourse.tile as tile
from concourse import bacc, bass_utils
from concourse.bass_interp import get_hw_module
from concourse.masks import make_identity

H = 16
DH = 64
WIN = 256
D = 1024
B = 4
N = 4096
N_CORES = 8
N_WIN_TOTAL = B * N // WIN          # 64
N_WIN = N_WIN_TOTAL // N_CORES      # 8 windows per core
TOK = N_WIN * WIN                   # 2048 token rows per core
SCALE = DH ** -0.5

F32 = mybir.dt.float32
F32R = mybir.dt.float32r


def _r(ap):
    return ap.bitcast(F32R)


def _body(tc, xq, wq, wk, wv, wo, out, n_win):
    nc = tc.nc
    from contextlib import ExitStack

    with ExitStack() as ctx:
        singles = ctx.enter_context(tc.tile_pool(name="singles", bufs=1))
        xpool = ctx.enter_context(tc.tile_pool(name="xpool", bufs=2))
        acts = ctx.enter_context(tc.tile_pool(name="acts", bufs=1))
        heads = ctx.enter_context(tc.tile_pool(name="heads", bufs=2))
        ypool = ctx.enter_context(tc.tile_pool(name="ypool", bufs=2))
        psA = ctx.enter_context(tc.tile_pool(name="psA", bufs=2, space="PSUM"))
        psS = ctx.enter_context(tc.tile_pool(name="psS", bufs=2, space="PSUM"))
        psV = ctx.enter_context(tc.tile_pool(name="psV", bufs=2, space="PSUM"))

        # ---- constants / weights (resident all kernel) ----
        ident_f = singles.tile([128, 128], F32)
        make_identity(nc, ident_f[:])
        ident = singles.tile([128, 128], F32R)
        nc.vector.tensor_copy(ident[:], ident_f[:])
        ones_f = singles.tile([128, 64], F32)
        nc.gpsimd.memset(ones_f[:], 1.0)
        ones64 = singles.tile([128, 64], F32R)
        nc.vector.tensor_copy(ones64[:], ones_f[:])

        # first window's X before the big weight DMAs so transposes start early
        x_first = [xpool.tile([128, D], F32R, tag="x", name=f"x0_{i}") for i in range(2)]
        for tt in range(2):
            nc.sync.dma_start(x_first[tt][:], xq[tt * 128:(tt + 1) * 128, :])

        wsb = {}
        for name, w in (("wq", wq), ("wk", wk), ("wv", wv), ("wo", wo)):
            t = singles.tile([128, 8 * D], F32R, tag=name, name=f"sb_{name}")
            for kt in range(8):
                nc.sync.dma_start(
                    t[:, kt * D:(kt + 1) * D], w[kt * 128:(kt + 1) * 128, :]
                )
            wsb[name] = t

        def emit_transposes(w, x_sb, xt):
            for dt_ in range(8):
                for tt in range(2):
                    pt = psA.tile([128, 128], F32R, tag="acc", name=f"pt_{w}_{dt_}_{tt}")
                    nc.tensor.transpose(
                        pt[:], x_sb[tt][:, dt_ * 128:(dt_ + 1) * 128], ident[:]
                    )
                    nc.vector.tensor_copy(
                        xt[:, dt_ * WIN + tt * 128:dt_ * WIN + tt * 128 + 128], pt[:]
                    )

        def emit_y_group(w, o2T, it, ec):
            row0 = w * WIN
            py = psA.tile([128, 512], F32, tag="acc", name=f"py_{w}_{it}_{ec}")
            for kt2 in range(8):
                nc.tensor.matmul(
                    py[:],
                    o2T[:, kt2 * WIN + it * 128:kt2 * WIN + (it + 1) * 128],
                    wsb["wo"][:, kt2 * D + ec * 512:kt2 * D + (ec + 1) * 512],
                    start=(kt2 == 0),
                    stop=(kt2 == 7),
                )
            y_sb = ypool.tile([128, 512], F32, tag="y", name=f"y_{w}_{it}_{ec}")
            nc.vector.tensor_copy(y_sb[:], py[:])
            nc.sync.dma_start(
                out[row0 + it * 128:row0 + (it + 1) * 128, ec * 512:(ec + 1) * 512],
                y_sb[:],
            )

        prev = None  # (o2T of previous window)
        for w in range(n_win):
            row0 = w * WIN
            if w == 0:
                x_sb = x_first
            else:
                x_sb = [xpool.tile([128, D], F32R, tag="x", name=f"x_{w}_{i}") for i in range(2)]
                for tt in range(2):
                    nc.sync.dma_start(
                        x_sb[tt][:], xq[row0 + tt * 128:row0 + (tt + 1) * 128, :]
                    )

            xt = acts.tile([128, 8 * WIN], F32R, tag="xt", name=f"xt_{w}")
            if prev is None:
                emit_transposes(w, x_sb, xt)
            else:
                # interleave: 4 transposes, then one Y group of previous window
                for chunk in range(4):
                    for dt_ in range(2 * chunk, 2 * chunk + 2):
                        for tt in range(2):
                            pt = psA.tile([128, 128], F32R, tag="acc",
                                          name=f"pt_{w}_{dt_}_{tt}")
                            nc.tensor.transpose(
                                pt[:], x_sb[tt][:, dt_ * 128:(dt_ + 1) * 128], ident[:]
                            )
                            nc.vector.tensor_copy(
                                xt[:, dt_ * WIN + tt * 128:dt_ * WIN + tt * 128 + 128],
                                pt[:],
                            )
                    emit_y_group(w - 1, prev, chunk // 2, chunk % 2)

            # ---- qT, kT [128, 2048] ----
            proj = {}
            for pname, wname in (("qT", "wq"), ("kT", "wk")):
                dst = acts.tile([128, 8 * WIN], F32R, tag=pname, name=f"{pname}_{w}")
                wtile = wsb[wname]
                for ot in range(8):
                    pq = psA.tile([128, WIN], F32, tag="acc", name=f"pq_{w}_{pname}_{ot}")
                    for kt in range(8):
                        nc.tensor.matmul(
                            pq[:],
                            wtile[:, kt * D + ot * 128:kt * D + (ot + 1) * 128],
                            xt[:, kt * WIN:(kt + 1) * WIN],
                            start=(kt == 0),
                            stop=(kt == 7),
                        )
                    nc.vector.tensor_copy(dst[:, ot * WIN:(ot + 1) * WIN], pq[:])
                proj[pname] = dst
            qT, kT = proj["qT"], proj["kT"]

            # ---- v natural [128 j, 2048] ----
            v_sb = acts.tile([128, 2 * D], F32R, tag="v", name=f"v_{w}")
            for jt in range(2):
                for oc in range(2):
                    pv = psA.tile([128, 512], F32, tag="acc", name=f"pv_{w}_{jt}_{oc}")
                    for kt in range(8):
                        nc.tensor.matmul(
                            pv[:],
                            xt[:, kt * WIN + jt * 128:kt * WIN + (jt + 1) * 128],
                            wsb["wv"][:, kt * D + oc * 512:kt * D + (oc + 1) * 512],
                            start=(kt == 0),
                            stop=(kt == 7),
                        )
                    nc.vector.tensor_copy(
                        v_sb[:, jt * D + oc * 512:jt * D + (oc + 1) * 512], pv[:]
                    )

            # ---- attention: head pairs, software-pipelined ----
            o2T = acts.tile([128, 8 * WIN], F32R, tag="o2T", name=f"o2T_{w}")

            es_t = [None] * H

            def emit_sim(h):
                prow = (h % 2) * 64
                ocol = (h // 2) * WIN
                qh = qT[prow:prow + 64, ocol:ocol + WIN]
                kh = kT[prow:prow + 64, ocol:ocol + WIN]
                ps_sim = psS.tile([128, 512], F32, tag="sim", name=f"sim_{w}_{h}")
                for jt in range(2):
                    nc.tensor.matmul(
                        ps_sim[:, jt * WIN:(jt + 1) * WIN],
                        kh[:, jt * 128:(jt + 1) * 128],
                        qh,
                        start=True,
                        stop=True,
                    )
                e = heads.tile([128, 512], F32R, tag="es", name=f"es_{w}_{h}")
                nc.scalar.activation(
                    e[:], ps_sim[:], mybir.ActivationFunctionType.Exp, scale=SCALE
                )
                es_t[h] = e

            def emit_pair(p):
                for h in (2 * p, 2 * p + 1):
                    s_ps = psV.tile([64, WIN], F32, tag="s", bufs=2,
                                    name=f"s_{w}_{h}")
                    av_ps = psV.tile([64, WIN], F32, tag="av", bufs=2,
                                     name=f"av_{w}_{h}")
                    for jt in range(2):
                        nc.tensor.matmul(
                            s_ps[:],
                            ones64[:, 0:64],
                            es_t[h][:, jt * WIN:(jt + 1) * WIN],
                            start=(jt == 0),
                            stop=(jt == 1),
                        )
                    for jt in range(2):
                        nc.tensor.matmul(
                            av_ps[:],
                            v_sb[:, jt * D + h * DH:jt * D + (h + 1) * DH],
                            es_t[h][:, jt * WIN:(jt + 1) * WIN],
                            start=(jt == 0),
                            stop=(jt == 1),
                        )
                    s_sb = heads.tile([64, WIN], F32, tag="s_sb",
                                      name=f"ssb_{w}_{h}")
                    nc.vector.tensor_copy(s_sb[:], s_ps[:])
                    rs = heads.tile([64, WIN], F32, tag="rs", name=f"rs_{w}_{h}")
                    nc.vector.reciprocal_approx_fast(rs[:], s_sb[:])
                    r0 = (h % 2) * 64
                    nc.vector.tensor_mul(
                        o2T[r0:r0 + 64, p * WIN:(p + 1) * WIN], av_ps[:], rs[:]
                    )
                    es_t[h] = None

            emit_sim(0)
            emit_sim(1)
            for p in range(1, 8):
                emit_sim(2 * p)
                emit_sim(2 * p + 1)
                emit_pair(p - 1)
            emit_pair(7)

            prev = o2T

        for chunk in range(4):
            emit_y_group(n_win - 1, prev, chunk // 2, chunk % 2)


_CACHE = {}


def _build(n_win=N_WIN):
    key = n_win
    if key in _CACHE:
        return _CACHE[key]
    tok = n_win * WIN
    nc = bacc.Bacc(
        "TRN2", target_bir_lowering=False, debug=False, num_devices=N_CORES
    )
    xq = nc.dram_tensor("xq", [tok, D], F32R, kind="ExternalInput").ap()
    wq = nc.dram_tensor("Wq", [D, D], F32R, kind="ExternalInput").ap()
    wk = nc.dram_tensor("Wk", [D, D], F32R, kind="ExternalInput").ap()
    wv = nc.dram_tensor("Wv", [D, D], F32R, kind="ExternalInput").ap()
    wo = nc.dram_tensor("Wo", [D, D], F32R, kind="ExternalInput").ap()
    out = nc.dram_tensor("out", [tok, D], F32, kind="ExternalOutput").ap()
    with tile.TileContext(nc) as tc:
        _body(tc, xq, wq, wk, wv, wo, out, n_win)
    nc.compile()
    nc.m = get_hw_module(nc.m)
    _CACHE[key] = nc
    return nc


def run(query, Wq, Wk, Wv, Wo, bo, n_win=N_WIN, **spmd_kwargs):
    nc = _build(n_win)
    tok = n_win * WIN
    q2 = np.ascontiguousarray(np.asarray(query, dtype=np.float32).reshape(-1, D))
    weights = {
        "Wq": np.ascontiguousarray(np.asarray(Wq, np.float32)),
        "Wk": np.ascontiguousarray(np.asarray(Wk, np.float32)),
        "Wv": np.ascontiguousarray(np.asarray(Wv, np.float32)),
        "Wo": np.ascontiguousarray(np.asarray(Wo, np.float32)),
    }
    in_maps = []
    for c in range(N_CORES):
        m = {"xq": q2[c * TOK:c * TOK + tok]}
        m.update(weights)
        in_maps.append(m)
    res = bass_utils.run_bass_kernel_spmd(
        nc, in_maps, core_ids=list(range(N_CORES)), **spmd_kwargs
    )
    outs = [res.results[c]["out"] for c in range(N_CORES)]
    return outs, res


def kernel(query, context, Wq, Wk, Wv, Wo, bo):
    outs, _ = run(query, Wq, Wk, Wv, Wo, bo)
    y = np.concatenate(outs, axis=0).reshape(B, N, D)
    bo = np.asarray(bo, np.float32)
    if bo.any():
        y = y + bo  # bias is structurally zero for this problem; host-add keeps exactness
    return y.astype(np.float32)



# revision 3
# speedup vs baseline: 1.3081x; 1.3081x over previous
"""Windowed local self-attention (CrossAttention module with the context-
overwrite bug faithfully reproduced) on 8 Trainium2 NeuronCores.

Full-input contract: kernel(**inputs) takes the unsharded tensors and
returns the full (4, 4096, 1024) output. Internally the 64 independent
windows of 256 tokens are data-parallel sharded 8-per-core; the four
projection weights are broadcast to every core. No collectives needed.

All matmul operands are bf16 (host-cast): 1 cycle/row on the PE at any
moving free-dim, half the SBUF/DMA traffic of fp32, and far less PE
power draw than fp32 HIGH mode (which triggered 50%-utilization
periodic throttling in the fp32r version of this kernel). PSUM
accumulation stays fp32; the softmax normalization and final output are
computed in fp32.

Per-core pipeline (window = 256 tokens, H=16 heads, DH=64):
  X  --PE transpose-->  XT [d, i]
  qT = Wq.T @ X.T   (lhsT=Wq tiles,  rhs=XT)          [o, i]
  kT = Wk.T @ X.T                                      [o, i]
  v  = X @ Wv       (lhsT=XT tiles,  rhs=Wv)           [j, o]
  per head h:
    simT = kT_h.T-free @ qT_h   -> [j, i] in PSUM     (j on partitions)
    es   = exp(0.125 * simT)    (ACT, PSUM->SBUF, bf16 out)
    S    = ones[j,64].T @ es    -> [64, i] broadcast row-sums (PE)
    rS   = 1/S                  (DVE reciprocal, straight from PSUM)
    o2u  = v_h.T-free @ es      -> [d, i] in PSUM      (AV matmul)
    o2T  = o2u * rS             (writes stacked bf16 [o, i] SBUF)
  Y = o2T.T @ Wo       (lhsT=o2T tiles, rhs=Wo; zero bias added host-side)
"""

import numpy as np
import ml_dtypes

import concourse.bass as bass
import concourse.mybir as mybir
import concourse.tile as tile
from concourse import bacc, bass_utils
from concourse.bass_interp import get_hw_module
from concourse.masks import make_identity

H = 16
DH = 64
WIN = 256
D = 1024
B = 4
N = 4096
N_CORES = 8
N_WIN_TOTAL = B * N // WIN          # 64
N_WIN = N_WIN_TOTAL // N_CORES      # 8 windows per core
TOK = N_WIN * WIN                   # 2048 token rows per core
SCALE = DH ** -0.5

F32 = mybir.dt.float32
BF16 = mybir.dt.bfloat16


def _body(tc, xq, wq, wk, wv, wo, out, n_win):
    nc = tc.nc
    from contextlib import ExitStack

    with ExitStack() as ctx:
        singles = ctx.enter_context(tc.tile_pool(name="singles", bufs=1))
        xpool = ctx.enter_context(tc.tile_pool(name="xpool", bufs=2))
        acts = ctx.enter_context(tc.tile_pool(name="acts", bufs=1))
        heads = ctx.enter_context(tc.tile_pool(name="heads", bufs=2))
        ypool = ctx.enter_context(tc.tile_pool(name="ypool", bufs=2))
        psA = ctx.enter_context(tc.tile_pool(name="psA", bufs=2, space="PSUM"))
        psS = ctx.enter_context(tc.tile_pool(name="psS", bufs=2, space="PSUM"))
        psV = ctx.enter_context(tc.tile_pool(name="psV", bufs=2, space="PSUM"))

        # ---- constants / weights (resident all kernel) ----
        ident_f = singles.tile([128, 128], F32)
        make_identity(nc, ident_f[:])
        ident = singles.tile([128, 128], BF16)
        nc.vector.tensor_copy(ident[:], ident_f[:])
        ones_f = singles.tile([128, 64], F32)
        nc.gpsimd.memset(ones_f[:], 1.0)
        ones64 = singles.tile([128, 64], BF16)
        nc.vector.tensor_copy(ones64[:], ones_f[:])

        # first window's X before the big weight DMAs so transposes start early
        x_first = [xpool.tile([128, D], BF16, tag="x", name=f"x0_{i}") for i in range(2)]
        for tt in range(2):
            nc.sync.dma_start(x_first[tt][:], xq[tt * 128:(tt + 1) * 128, :])

        wsb = {}
        for name, w in (("wq", wq), ("wk", wk), ("wv", wv), ("wo", wo)):
            t = singles.tile([128, 8 * D], BF16, tag=name, name=f"sb_{name}")
            for kt in range(8):
                nc.sync.dma_start(
                    t[:, kt * D:(kt + 1) * D], w[kt * 128:(kt + 1) * 128, :]
                )
            wsb[name] = t

        def emit_transposes(w, x_sb, xt):
            for dt_ in range(8):
                for tt in range(2):
                    pt = psA.tile([128, 128], BF16, tag="acc", name=f"pt_{w}_{dt_}_{tt}")
                    nc.tensor.transpose(
                        pt[:], x_sb[tt][:, dt_ * 128:(dt_ + 1) * 128], ident[:]
                    )
                    nc.vector.tensor_copy(
                        xt[:, dt_ * WIN + tt * 128:dt_ * WIN + tt * 128 + 128], pt[:]
                    )

        def emit_y_group(w, o2T, it, ec):
            row0 = w * WIN
            py = psA.tile([128, 512], F32, tag="acc", name=f"py_{w}_{it}_{ec}")
            for kt2 in range(8):
                nc.tensor.matmul(
                    py[:],
                    o2T[:, kt2 * WIN + it * 128:kt2 * WIN + (it + 1) * 128],
                    wsb["wo"][:, kt2 * D + ec * 512:kt2 * D + (ec + 1) * 512],
                    start=(kt2 == 0),
                    stop=(kt2 == 7),
                )
            y_sb = ypool.tile([128, 512], F32, tag="y", name=f"y_{w}_{it}_{ec}")
            nc.vector.tensor_copy(y_sb[:], py[:])
            nc.sync.dma_start(
                out[row0 + it * 128:row0 + (it + 1) * 128, ec * 512:(ec + 1) * 512],
                y_sb[:],
            )

        prev = None  # (o2T of previous window)
        for w in range(n_win):
            row0 = w * WIN
            if w == 0:
                x_sb = x_first
            else:
                x_sb = [xpool.tile([128, D], BF16, tag="x", name=f"x_{w}_{i}") for i in range(2)]
                for tt in range(2):
                    nc.sync.dma_start(
                        x_sb[tt][:], xq[row0 + tt * 128:row0 + (tt + 1) * 128, :]
                    )

            xt = acts.tile([128, 8 * WIN], BF16, tag="xt", name=f"xt_{w}")
            if prev is None:
                emit_transposes(w, x_sb, xt)
            else:
                # interleave: 4 transposes, then one Y group of previous window
                for chunk in range(4):
                    for dt_ in range(2 * chunk, 2 * chunk + 2):
                        for tt in range(2):
                            pt = psA.tile([128, 128], BF16, tag="acc",
                                          name=f"pt_{w}_{dt_}_{tt}")
                            nc.tensor.transpose(
                                pt[:], x_sb[tt][:, dt_ * 128:(dt_ + 1) * 128], ident[:]
                            )
                            nc.vector.tensor_copy(
                                xt[:, dt_ * WIN + tt * 128:dt_ * WIN + tt * 128 + 128],
                                pt[:],
                            )
                    emit_y_group(w - 1, prev, chunk // 2, chunk % 2)

            # ---- qT, kT [128, 2048] ----
            proj = {}
            for pname, wname in (("qT", "wq"), ("kT", "wk")):
                dst = acts.tile([128, 8 * WIN], BF16, tag=pname, name=f"{pname}_{w}")
                wtile = wsb[wname]
                for ot in range(8):
                    pq = psA.tile([128, WIN], F32, tag="acc", name=f"pq_{w}_{pname}_{ot}")
                    for kt in range(8):
                        nc.tensor.matmul(
                            pq[:],
                            wtile[:, kt * D + ot * 128:kt * D + (ot + 1) * 128],
                            xt[:, kt * WIN:(kt + 1) * WIN],
                            start=(kt == 0),
                            stop=(kt == 7),
                        )
                    nc.vector.tensor_copy(dst[:, ot * WIN:(ot + 1) * WIN], pq[:])
                proj[pname] = dst
            qT, kT = proj["qT"], proj["kT"]

            # ---- v natural [128 j, 2048] ----
            v_sb = acts.tile([128, 2 * D], BF16, tag="v", name=f"v_{w}")
            for jt in range(2):
                for oc in range(2):
                    pv = psA.tile([128, 512], F32, tag="acc", name=f"pv_{w}_{jt}_{oc}")
                    for kt in range(8):
                        nc.tensor.matmul(
                            pv[:],
                            xt[:, kt * WIN + jt * 128:kt * WIN + (jt + 1) * 128],
                            wsb["wv"][:, kt * D + oc * 512:kt * D + (oc + 1) * 512],
                            start=(kt == 0),
                            stop=(kt == 7),
                        )
                    nc.scalar.copy(
                        v_sb[:, jt * D + oc * 512:jt * D + (oc + 1) * 512], pv[:]
                    )

            # ---- attention: head pairs, software-pipelined ----
            o2T = acts.tile([128, 8 * WIN], BF16, tag="o2T", name=f"o2T_{w}")

            es_t = [None] * H

            def emit_sim(h):
                prow = (h % 2) * 64
                ocol = (h // 2) * WIN
                qh = qT[prow:prow + 64, ocol:ocol + WIN]
                kh = kT[prow:prow + 64, ocol:ocol + WIN]
                ps_sim = psS.tile([128, 512], F32, tag="sim", name=f"sim_{w}_{h}")
                for jt in range(2):
                    nc.tensor.matmul(
                        ps_sim[:, jt * WIN:(jt + 1) * WIN],
                        kh[:, jt * 128:(jt + 1) * 128],
                        qh,
                        start=True,
                        stop=True,
                    )
                e = heads.tile([128, 512], BF16, tag="es", name=f"es_{w}_{h}")
                nc.scalar.activation(
                    e[:], ps_sim[:], mybir.ActivationFunctionType.Exp, scale=SCALE
                )
                es_t[h] = e

            def emit_pair(p):
                for h in (2 * p, 2 * p + 1):
                    s_ps = psV.tile([64, WIN], F32, tag="s", bufs=2,
                                    name=f"s_{w}_{h}")
                    av_ps = psV.tile([64, WIN], F32, tag="av", bufs=2,
                                     name=f"av_{w}_{h}")
                    for jt in range(2):
                        nc.tensor.matmul(
                            s_ps[:],
                            ones64[:, 0:64],
                            es_t[h][:, jt * WIN:(jt + 1) * WIN],
                            start=(jt == 0),
                            stop=(jt == 1),
                        )
                    for jt in range(2):
                        nc.tensor.matmul(
                            av_ps[:],
                            v_sb[:, jt * D + h * DH:jt * D + (h + 1) * DH],
                            es_t[h][:, jt * WIN:(jt + 1) * WIN],
                            start=(jt == 0),
                            stop=(jt == 1),
                        )
                    rs = heads.tile([64, WIN], F32, tag="rs", name=f"rs_{w}_{h}")
                    nc.vector.reciprocal_approx_fast(rs[:], s_ps[:])
                    r0 = (h % 2) * 64
                    nc.vector.tensor_mul(
                        o2T[r0:r0 + 64, p * WIN:(p + 1) * WIN], av_ps[:], rs[:]
                    )
                    es_t[h] = None

            emit_sim(0)
            emit_sim(1)
            for p in range(1, 8):
                emit_sim(2 * p)
                emit_sim(2 * p + 1)
                emit_pair(p - 1)
            emit_pair(7)

            prev = o2T

        for chunk in range(4):
            emit_y_group(n_win - 1, prev, chunk // 2, chunk % 2)


_CACHE = {}


def _build(n_win=N_WIN):
    key = n_win
    if key in _CACHE:
        return _CACHE[key]
    tok = n_win * WIN
    nc = bacc.Bacc(
        "TRN2", target_bir_lowering=False, debug=False, num_devices=N_CORES
    )
    xq = nc.dram_tensor("xq", [tok, D], BF16, kind="ExternalInput").ap()
    wq = nc.dram_tensor("Wq", [D, D], BF16, kind="ExternalInput").ap()
    wk = nc.dram_tensor("Wk", [D, D], BF16, kind="ExternalInput").ap()
    wv = nc.dram_tensor("Wv", [D, D], BF16, kind="ExternalInput").ap()
    wo = nc.dram_tensor("Wo", [D, D], BF16, kind="ExternalInput").ap()
    out = nc.dram_tensor("out", [tok, D], F32, kind="ExternalOutput").ap()
    with tile.TileContext(nc) as tc:
        _body(tc, xq, wq, wk, wv, wo, out, n_win)
    nc.compile()
    nc.m = get_hw_module(nc.m)
    _CACHE[key] = nc
    return nc


def run(query, Wq, Wk, Wv, Wo, bo, n_win=N_WIN, **spmd_kwargs):
    nc = _build(n_win)
    tok = n_win * WIN
    bf = ml_dtypes.bfloat16
    q2 = np.ascontiguousarray(
        np.asarray(query, dtype=np.float32).reshape(-1, D).astype(bf)
    )
    weights = {
        "Wq": np.ascontiguousarray(np.asarray(Wq, np.float32).astype(bf)),
        "Wk": np.ascontiguousarray(np.asarray(Wk, np.float32).astype(bf)),
        "Wv": np.ascontiguousarray(np.asarray(Wv, np.float32).astype(bf)),
        "Wo": np.ascontiguousarray(np.asarray(Wo, np.float32).astype(bf)),
    }
    in_maps = []
    for c in range(N_CORES):
        m = {"xq": q2[c * TOK:c * TOK + tok]}
        m.update(weights)
        in_maps.append(m)
    res = bass_utils.run_bass_kernel_spmd(
        nc, in_maps, core_ids=list(range(N_CORES)), **spmd_kwargs
    )
    outs = [res.results[c]["out"] for c in range(N_CORES)]
    return outs, res


def kernel(query, context, Wq, Wk, Wv, Wo, bo):
    outs, _ = run(query, Wq, Wk, Wv, Wo, bo)
    y = np.concatenate(outs, axis=0).reshape(B, N, D)
    bo = np.asarray(bo, np.float32)
    if bo.any():
        y = y + bo  # bias is structurally zero for this problem; host-add keeps exactness
    return y.astype(np.float32)


# revision 7
# speedup vs baseline: 1.4011x; 1.0711x over previous
"""Windowed local self-attention (CrossAttention module with the context-
overwrite bug faithfully reproduced) on 8 Trainium2 NeuronCores.

Full-input contract: kernel(**inputs) takes the unsharded tensors and
returns the full (4, 4096, 1024) output. Internally the 64 independent
windows of 256 tokens are data-parallel sharded 8-per-core; the four
projection weights are broadcast to every core. No collectives needed.

All matmul operands are bf16 (host-cast): 1 cycle/row on the PE at any
moving free-dim, half the SBUF/DMA traffic of fp32, and far less PE
power draw than fp32 HIGH mode (which triggered 50%-utilization
periodic throttling in the fp32r version of this kernel). PSUM
accumulation stays fp32; the softmax normalization and final output are
computed in fp32.

Per-core pipeline (window = 256 tokens, H=16 heads, DH=64):
  X  --PE transpose-->  XT [d, i]
  qT = Wq.T @ X.T   (lhsT=Wq tiles,  rhs=XT)          [o, i]
  kT = Wk.T @ X.T                                      [o, i]
  v  = X @ Wv       (lhsT=XT tiles,  rhs=Wv)           [j, o]
  per head h:
    simT = kT_h.T-free @ qT_h   -> [j, i] in PSUM     (j on partitions)
    es   = exp(0.125 * simT)    (ACT, PSUM->SBUF, bf16 out)
    S    = ones[j,64].T @ es    -> [64, i] broadcast row-sums (PE)
    rS   = 1/S                  (DVE reciprocal, straight from PSUM)
    o2u  = v_h.T-free @ es      -> [d, i] in PSUM      (AV matmul)
    o2T  = o2u * rS             (writes stacked bf16 [o, i] SBUF)
  Y = o2T.T @ Wo       (lhsT=o2T tiles, rhs=Wo; zero bias added host-side)
"""

import numpy as np
import ml_dtypes

import concourse.bass as bass
import concourse.mybir as mybir
import concourse.tile as tile
from concourse import bacc, bass_utils
from concourse.bass_interp import get_hw_module
from concourse.masks import make_identity

H = 16
DH = 64
WIN = 256
D = 1024
B = 4
N = 4096
N_CORES = 8
N_WIN_TOTAL = B * N // WIN          # 64
N_WIN = N_WIN_TOTAL // N_CORES      # 8 windows per core
TOK = N_WIN * WIN                   # 2048 token rows per core
SCALE = DH ** -0.5

F32 = mybir.dt.float32
BF16 = mybir.dt.bfloat16


def _body(tc, xq, wq, wk, wv, wo, out, n_win):
    nc = tc.nc
    from contextlib import ExitStack

    with ExitStack() as ctx:
        singles = ctx.enter_context(tc.tile_pool(name="singles", bufs=1))
        xpool = ctx.enter_context(tc.tile_pool(name="xpool", bufs=2))
        acts = ctx.enter_context(tc.tile_pool(name="acts", bufs=1))
        heads = ctx.enter_context(tc.tile_pool(name="heads", bufs=2))
        ypool = ctx.enter_context(tc.tile_pool(name="ypool", bufs=2))
        psA = ctx.enter_context(tc.tile_pool(name="psA", bufs=2, space="PSUM"))
        psS = ctx.enter_context(tc.tile_pool(name="psS", bufs=2, space="PSUM"))
        psV = ctx.enter_context(tc.tile_pool(name="psV", bufs=2, space="PSUM"))

        # ---- constants / weights (resident all kernel) ----
        ident_f = singles.tile([128, 128], F32)
        make_identity(nc, ident_f[:])
        ident = singles.tile([128, 128], BF16)
        nc.vector.tensor_copy(ident[:], ident_f[:])
        ones_f = singles.tile([128, 64], F32)
        nc.gpsimd.memset(ones_f[:], 1.0)
        ones64 = singles.tile([128, 64], BF16)
        nc.vector.tensor_copy(ones64[:], ones_f[:])

        # first window's X before the big weight DMAs so transposes start early
        x_first = [xpool.tile([128, D], BF16, tag="x", name=f"x0_{i}") for i in range(2)]
        for tt in range(2):
            nc.sync.dma_start(x_first[tt][:], xq[tt * 128:(tt + 1) * 128, :])

        wsb = {}
        for name, w in (("wq", wq), ("wk", wk), ("wv", wv), ("wo", wo)):
            t = singles.tile([128, 8 * D], BF16, tag=name, name=f"sb_{name}")
            for kt in range(8):
                nc.sync.dma_start(
                    t[:, kt * D:(kt + 1) * D], w[kt * 128:(kt + 1) * 128, :]
                )
            wsb[name] = t

        def emit_transposes(w, x_sb, xt):
            # 8 transposes batched per full PSUM bank -> one wide DVE copy
            for half in range(2):
                pt = psA.tile([128, 1024], BF16, tag="acc", name=f"pt_{w}_{half}")
                for d2 in range(4):
                    dt_ = half * 4 + d2
                    for tt in range(2):
                        nc.tensor.transpose(
                            pt[:, d2 * WIN + tt * 128:d2 * WIN + tt * 128 + 128],
                            x_sb[tt][:, dt_ * 128:(dt_ + 1) * 128],
                            ident[:],
                        )
                nc.vector.tensor_copy(
                    xt[:, half * 1024:(half + 1) * 1024], pt[:]
                )

        def emit_y_group(w, o2T, it, ec):
            row0 = w * WIN
            py = psA.tile([128, 512], F32, tag="acc", name=f"py_{w}_{it}_{ec}")
            for kt2 in range(8):
                nc.tensor.matmul(
                    py[:],
                    o2T[:, kt2 * WIN + it * 128:kt2 * WIN + (it + 1) * 128],
                    wsb["wo"][:, kt2 * D + ec * 512:kt2 * D + (ec + 1) * 512],
                    start=(kt2 == 0),
                    stop=(kt2 == 7),
                )
            y_sb = ypool.tile([128, 512], F32, tag="y", name=f"y_{w}_{it}_{ec}")
            nc.vector.tensor_copy(y_sb[:], py[:])
            nc.sync.dma_start(
                out[row0 + it * 128:row0 + (it + 1) * 128, ec * 512:(ec + 1) * 512],
                y_sb[:],
            )

        prev = None  # (o2T of previous window)
        for w in range(n_win):
            row0 = w * WIN
            if w == 0:
                x_sb = x_first
            else:
                x_sb = [xpool.tile([128, D], BF16, tag="x", name=f"x_{w}_{i}") for i in range(2)]
                for tt in range(2):
                    nc.sync.dma_start(
                        x_sb[tt][:], xq[row0 + tt * 128:row0 + (tt + 1) * 128, :]
                    )

            xt = acts.tile([128, 8 * WIN], BF16, tag="xt", name=f"xt_{w}")
            if prev is None:
                emit_transposes(w, x_sb, xt)
            else:
                # interleave: 4 transposes, then one Y group of previous window
                pt = None
                for chunk in range(4):
                    half = chunk // 2
                    if chunk % 2 == 0:
                        pt = psA.tile([128, 1024], BF16, tag="acc",
                                      name=f"pt_{w}_{half}")
                    for dt_ in range(2 * chunk, 2 * chunk + 2):
                        d2 = dt_ - half * 4
                        for tt in range(2):
                            nc.tensor.transpose(
                                pt[:, d2 * WIN + tt * 128:d2 * WIN + tt * 128 + 128],
                                x_sb[tt][:, dt_ * 128:(dt_ + 1) * 128],
                                ident[:],
                            )
                    if chunk % 2 == 1:
                        nc.vector.tensor_copy(
                            xt[:, half * 1024:(half + 1) * 1024], pt[:]
                        )
                    emit_y_group(w - 1, prev, chunk // 2, chunk % 2)

            # ---- qT, kT [128, 2048] ----
            proj = {}
            for pname, wname in (("qT", "wq"), ("kT", "wk")):
                dst = acts.tile([128, 8 * WIN], BF16, tag=pname, name=f"{pname}_{w}")
                wtile = wsb[wname]
                for og in range(4):  # 2 ot blocks share one full PSUM bank
                    pq = psA.tile([128, 2 * WIN], F32, tag="acc",
                                  name=f"pq_{w}_{pname}_{og}")
                    for oh in range(2):
                        ot = og * 2 + oh
                        for kt in range(8):
                            nc.tensor.matmul(
                                pq[:, oh * WIN:(oh + 1) * WIN],
                                wtile[:, kt * D + ot * 128:kt * D + (ot + 1) * 128],
                                xt[:, kt * WIN:(kt + 1) * WIN],
                                start=(kt == 0),
                                stop=(kt == 7),
                            )
                    nc.vector.tensor_copy(
                        dst[:, og * 2 * WIN:(og + 1) * 2 * WIN], pq[:]
                    )
                proj[pname] = dst
            qT, kT = proj["qT"], proj["kT"]

            # ---- v natural [128 j, 2048] ----
            v_sb = acts.tile([128, 2 * D], BF16, tag="v", name=f"v_{w}")
            for jt in range(2):
                for oc in range(2):
                    pv = psA.tile([128, 512], F32, tag="acc", name=f"pv_{w}_{jt}_{oc}")
                    for kt in range(8):
                        nc.tensor.matmul(
                            pv[:],
                            xt[:, kt * WIN + jt * 128:kt * WIN + (jt + 1) * 128],
                            wsb["wv"][:, kt * D + oc * 512:kt * D + (oc + 1) * 512],
                            start=(kt == 0),
                            stop=(kt == 7),
                        )
                    nc.scalar.copy(
                        v_sb[:, jt * D + oc * 512:jt * D + (oc + 1) * 512], pv[:]
                    )

            # ---- attention: head pairs, software-pipelined ----
            o2T = acts.tile([128, 8 * WIN], BF16, tag="o2T", name=f"o2T_{w}")

            es_t = [None] * H

            def emit_sim(h):
                prow = (h % 2) * 64
                ocol = (h // 2) * WIN
                qh = qT[prow:prow + 64, ocol:ocol + WIN]
                kh = kT[prow:prow + 64, ocol:ocol + WIN]
                ps_sim = psS.tile([128, 512], F32, tag="sim", name=f"sim_{w}_{h}")
                for jt in range(2):
                    nc.tensor.matmul(
                        ps_sim[:, jt * WIN:(jt + 1) * WIN],
                        kh[:, jt * 128:(jt + 1) * 128],
                        qh,
                        start=True,
                        stop=True,
                    )
                e = heads.tile([128, 512], BF16, tag="es", name=f"es_{w}_{h}")
                nc.scalar.activation(
                    e[:], ps_sim[:], mybir.ActivationFunctionType.Exp, scale=SCALE
                )
                es_t[h] = e

            def emit_pair(p):
                # both heads of the pair packed on partitions: head 2p on
                # rows 0-63, head 2p+1 on rows 64-127 -> one recip + one mul
                s2 = psV.tile([128, WIN], F32, tag="s", bufs=2, name=f"s_{w}_{p}")
                av2 = psV.tile([128, WIN], F32, tag="av", bufs=2, name=f"av_{w}_{p}")
                for hh in range(2):
                    h = 2 * p + hh
                    r0 = hh * 64
                    for jt in range(2):
                        nc.tensor.matmul(
                            s2[r0:r0 + 64, :],
                            ones64[:, 0:64],
                            es_t[h][:, jt * WIN:(jt + 1) * WIN],
                            start=(jt == 0),
                            stop=(jt == 1),
                        )
                    for jt in range(2):
                        nc.tensor.matmul(
                            av2[r0:r0 + 64, :],
                            v_sb[:, jt * D + h * DH:jt * D + (h + 1) * DH],
                            es_t[h][:, jt * WIN:(jt + 1) * WIN],
                            start=(jt == 0),
                            stop=(jt == 1),
                        )
                rs = heads.tile([128, WIN], F32, tag="rs", name=f"rs_{w}_{p}")
                nc.vector.reciprocal_approx_fast(rs[:], s2[:])
                nc.vector.tensor_mul(
                    o2T[:, p * WIN:(p + 1) * WIN], av2[:], rs[:]
                )
                es_t[2 * p] = None
                es_t[2 * p + 1] = None

            emit_sim(0)
            emit_sim(1)
            for p in range(1, 8):
                emit_sim(2 * p)
                emit_sim(2 * p + 1)
                emit_pair(p - 1)
            emit_pair(7)

            prev = o2T

        for chunk in range(4):
            emit_y_group(n_win - 1, prev, chunk // 2, chunk % 2)


_CACHE = {}


def _build(n_win=N_WIN):
    key = n_win
    if key in _CACHE:
        return _CACHE[key]
    tok = n_win * WIN
    nc = bacc.Bacc(
        "TRN2", target_bir_lowering=False, debug=False, num_devices=N_CORES
    )
    xq = nc.dram_tensor("xq", [tok, D], BF16, kind="ExternalInput").ap()
    wq = nc.dram_tensor("Wq", [D, D], BF16, kind="ExternalInput").ap()
    wk = nc.dram_tensor("Wk", [D, D], BF16, kind="ExternalInput").ap()
    wv = nc.dram_tensor("Wv", [D, D], BF16, kind="ExternalInput").ap()
    wo = nc.dram_tensor("Wo", [D, D], BF16, kind="ExternalInput").ap()
    out = nc.dram_tensor("out", [tok, D], F32, kind="ExternalOutput").ap()
    with tile.TileContext(nc) as tc:
        _body(tc, xq, wq, wk, wv, wo, out, n_win)
    nc.compile()
    nc.m = get_hw_module(nc.m)
    _CACHE[key] = nc
    return nc


def run(query, Wq, Wk, Wv, Wo, bo, n_win=N_WIN, **spmd_kwargs):
    nc = _build(n_win)
    tok = n_win * WIN
    bf = ml_dtypes.bfloat16
    q2 = np.ascontiguousarray(
        np.asarray(query, dtype=np.float32).reshape(-1, D).astype(bf)
    )
    weights = {
        "Wq": np.ascontiguousarray(np.asarray(Wq, np.float32).astype(bf)),
        "Wk": np.ascontiguousarray(np.asarray(Wk, np.float32).astype(bf)),
        "Wv": np.ascontiguousarray(np.asarray(Wv, np.float32).astype(bf)),
        "Wo": np.ascontiguousarray(np.asarray(Wo, np.float32).astype(bf)),
    }
    in_maps = []
    for c in range(N_CORES):
        m = {"xq": q2[c * TOK:c * TOK + tok]}
        m.update(weights)
        in_maps.append(m)
    res = bass_utils.run_bass_kernel_spmd(
        nc, in_maps, core_ids=list(range(N_CORES)), **spmd_kwargs
    )
    outs = [res.results[c]["out"] for c in range(N_CORES)]
    return outs, res


def kernel(query, context, Wq, Wk, Wv, Wo, bo):
    outs, _ = run(query, Wq, Wk, Wv, Wo, bo)
    y = np.concatenate(outs, axis=0).reshape(B, N, D)
    bo = np.asarray(bo, np.float32)
    if bo.any():
        y = y + bo  # bias is structurally zero for this problem; host-add keeps exactness
    return y.astype(np.float32)


# revision 11
# speedup vs baseline: 1.4578x; 1.0405x over previous
"""Windowed local self-attention (CrossAttention module with the context-
overwrite bug faithfully reproduced) on 8 Trainium2 NeuronCores.

Full-input contract: kernel(**inputs) takes the unsharded tensors and
returns the full (4, 4096, 1024) output. Internally the 64 independent
windows of 256 tokens are data-parallel sharded 8-per-core; the four
projection weights are broadcast to every core. No collectives needed.

All matmul operands are bf16 (host-cast): 1 cycle/row on the PE, half
the SBUF/DMA traffic of fp32, and far less PE power draw than fp32 HIGH
mode (which triggered 50%-utilization periodic throttling in the fp32r
version). PSUM accumulation, softmax normalization and the final output
stay fp32.

Windows are processed in PAIRS (512 tokens) so every projection /
output matmul streams the maximum 512 moving rows per instruction,
hiding LDWEIGHTS under the row stream.

Per-core pipeline (window = 256 tokens, H=16 heads, DH=64):
  X  --PE transpose-->  XT [d, i]            (8 transposes per PSUM bank)
  qT = Wq.T @ X.T   (lhsT=Wq tiles,  rhs=XT)          [o, i]
  kT = Wk.T @ X.T                                      [o, i]
  v  = X @ Wv       (lhsT=XT tiles,  rhs=Wv)           [j, v|1]
       stored interleaved per head as [v_h (64) | ones (64)] so that
  per head h (per window):
    simT = kT_h.T-free @ qT_h   -> [j, i] in PSUM     (j on partitions)
    es   = exp(0.125 * simT)    (ACT, PSUM->SBUF, bf16)
    av   = [v_h|1].T-free @ es  -> [128, i] PSUM: rows 0-63 = o2u_h,
           rows 64-127 = column sums S_h (replicated) -- the softmax
           denominator comes free out of the AV matmul, no S matmul.
    rS   = 1/S                  (DVE reciprocal from PSUM rows 64:128)
    o2T  = o2u * rS             (DVE, bf16 [o, i] SBUF)
  Y = o2T.T @ Wo       (lhsT=o2T tiles, rhs=Wo; zero bias added host-side)
"""

import numpy as np
import ml_dtypes

import concourse.bass as bass
import concourse.mybir as mybir
import concourse.tile as tile
from concourse import bacc, bass_utils
from concourse.bass_interp import get_hw_module
from concourse.masks import make_identity

H = 16
DH = 64
WIN = 256
D = 1024
B = 4
N = 4096
N_CORES = 8
N_WIN_TOTAL = B * N // WIN          # 64
N_WIN = N_WIN_TOTAL // N_CORES      # 8 windows per core
TOK = N_WIN * WIN                   # 2048 token rows per core
PAIR = 2 * WIN                      # 512 tokens per window pair
SCALE = DH ** -0.5

F32 = mybir.dt.float32
BF16 = mybir.dt.bfloat16


def _body(tc, xq, wq, wk, wv, wo, out, n_win):
    nc = tc.nc
    from contextlib import ExitStack

    n_pair = n_win // 2

    with ExitStack() as ctx:
        singles = ctx.enter_context(tc.tile_pool(name="singles", bufs=1))
        xpool = ctx.enter_context(tc.tile_pool(name="xpool", bufs=2))
        acts = ctx.enter_context(tc.tile_pool(name="acts", bufs=1))
        heads = ctx.enter_context(tc.tile_pool(name="heads", bufs=3))
        ypool = ctx.enter_context(tc.tile_pool(name="ypool", bufs=2))
        psA = ctx.enter_context(tc.tile_pool(name="psA", bufs=2, space="PSUM"))
        psS = ctx.enter_context(tc.tile_pool(name="psS", bufs=3, space="PSUM"))
        psV = ctx.enter_context(tc.tile_pool(name="psV", bufs=3, space="PSUM"))

        # ---- constants / weights (resident all kernel) ----
        ident_f = singles.tile([128, 128], F32)
        make_identity(nc, ident_f[:])
        ident = singles.tile([128, 128], BF16)
        nc.vector.tensor_copy(ident[:], ident_f[:])

        # first pair's X before the big weight DMAs so transposes start early
        x_first = [xpool.tile([128, D], BF16, tag="x", bufs=8, name=f"x0_{i}")
                   for i in range(4)]
        for tt in range(4):
            nc.sync.dma_start(x_first[tt][:], xq[tt * 128:(tt + 1) * 128, :])

        wsb = {}
        for name, w in (("wq", wq), ("wk", wk), ("wv", wv), ("wo", wo)):
            t = singles.tile([128, 8 * D], BF16, tag=name, name=f"sb_{name}")
            for kt in range(8):
                nc.sync.dma_start(
                    t[:, kt * D:(kt + 1) * D], w[kt * 128:(kt + 1) * 128, :]
                )
            wsb[name] = t

        # v double-buffer: window w uses v2b[w % 2]; per-head layout
        # [v_h (64 cols) | ones (64 cols)] so AV' yields sums on rows 64+.
        v2b = []
        for i in range(2):
            t = singles.tile([128, 2 * H * 128], BF16, name=f"v2_{i}")
            ones_view = t[:].rearrange("p (j h c) -> p j h c", j=2, h=H)[:, :, :, DH:]
            nc.gpsimd.memset(ones_view, 1.0)
            v2b.append(t)

        def emit_tp_group(wp, x_sb, xt, g):
            # one PSUM bank: 8 transposes (dt = 2g, 2g+1) -> one DVE copy
            pt = psA.tile([128, 1024], BF16, tag="acc", name=f"pt_{wp}_{g}")
            for d2 in range(2):
                dt_ = 2 * g + d2
                for tt in range(4):
                    nc.tensor.transpose(
                        pt[:, d2 * 512 + tt * 128:d2 * 512 + (tt + 1) * 128],
                        x_sb[tt][:, dt_ * 128:(dt_ + 1) * 128],
                        ident[:],
                    )
            nc.vector.tensor_copy(xt[:, 2 * g * 512:(2 * g + 2) * 512], pt[:])

        def emit_y_group(wp, o2T, it, ec):
            row0 = wp * PAIR
            py = psA.tile([128, 512], F32, tag="acc", name=f"py_{wp}_{it}_{ec}")
            for kt2 in range(8):
                nc.tensor.matmul(
                    py[:],
                    o2T[:, kt2 * 512 + it * 128:kt2 * 512 + (it + 1) * 128],
                    wsb["wo"][:, kt2 * D + ec * 512:kt2 * D + (ec + 1) * 512],
                    start=(kt2 == 0),
                    stop=(kt2 == 7),
                )
            y_sb = ypool.tile([128, 512], F32, tag="y", name=f"y_{wp}_{it}_{ec}")
            nc.vector.tensor_copy(y_sb[:], py[:])
            nc.sync.dma_start(
                out[row0 + it * 128:row0 + (it + 1) * 128, ec * 512:(ec + 1) * 512],
                y_sb[:],
            )

        prev = None  # o2T of previous pair
        for wp in range(n_pair):
            row0 = wp * PAIR
            if wp == 0:
                x_sb = x_first
            else:
                x_sb = [xpool.tile([128, D], BF16, tag="x", bufs=8,
                                   name=f"x_{wp}_{i}")
                        for i in range(4)]
                for tt in range(4):
                    nc.sync.dma_start(
                        x_sb[tt][:], xq[row0 + tt * 128:row0 + (tt + 1) * 128, :]
                    )

            xt = acts.tile([128, 8 * 512], BF16, tag="xt", bufs=2, name=f"xt_{wp}")
            for g in range(4):
                emit_tp_group(wp, x_sb, xt, g)
                if prev is not None:
                    emit_y_group(wp - 1, prev, g, 0)
                    emit_y_group(wp - 1, prev, g, 1)

            # ---- qT, kT [128 o, 8*512 i] ----
            proj = {}
            for pname, wname in (("qT", "wq"), ("kT", "wk")):
                dst = acts.tile([128, 8 * 512], BF16, tag=pname, bufs=2,
                                name=f"{pname}_{wp}")
                wtile = wsb[wname]
                for ot in range(8):
                    pq = psA.tile([128, 512], F32, tag="acc",
                                  name=f"pq_{wp}_{pname}_{ot}")
                    for kt in range(8):
                        nc.tensor.matmul(
                            pq[:],
                            wtile[:, kt * D + ot * 128:kt * D + (ot + 1) * 128],
                            xt[:, kt * 512:(kt + 1) * 512],
                            start=(kt == 0),
                            stop=(kt == 7),
                        )
                    nc.vector.tensor_copy(dst[:, ot * 512:(ot + 1) * 512], pq[:])
                proj[pname] = dst
            qT, kT = proj["qT"], proj["kT"]

            # ---- v [128 j, (jt, h, v|1)] strided into v2b buffers ----
            for tt in range(4):
                wl, jt = tt // 2, tt % 2
                vdst = v2b[wl]
                for oc in range(2):
                    pv = psA.tile([128, 512], F32, tag="acc",
                                  name=f"pv_{wp}_{tt}_{oc}")
                    for kt in range(8):
                        nc.tensor.matmul(
                            pv[:],
                            xt[:, kt * 512 + tt * 128:kt * 512 + (tt + 1) * 128],
                            wsb["wv"][:, kt * D + oc * 512:kt * D + (oc + 1) * 512],
                            start=(kt == 0),
                            stop=(kt == 7),
                        )
                    dsl = vdst[:, jt * H * 128 + oc * 8 * 128:
                               jt * H * 128 + (oc + 1) * 8 * 128]
                    nc.scalar.copy(
                        dsl.rearrange("p (h c) -> p h c", h=8)[:, :, 0:DH], pv[:]
                    )

            # ---- attention: 32 (window, head) steps, software-pipelined ----
            o2T = acts.tile([128, 8 * 512], BF16, tag="o2T", bufs=2,
                            name=f"o2T_{wp}")
            steps = [(wl, h) for wl in range(2) for h in range(H)]
            es_t = {}

            def emit_sim(step):
                wl, h = step
                prow = (h % 2) * 64
                ocol = (h // 2) * 512 + wl * WIN
                qh = qT[prow:prow + 64, ocol:ocol + WIN]
                kh = kT[prow:prow + 64, ocol:ocol + WIN]
                ps_sim = psS.tile([128, 512], F32, tag="sim",
                                  name=f"sim_{wp}_{wl}_{h}")
                for jt in range(2):
                    nc.tensor.matmul(
                        ps_sim[:, jt * WIN:(jt + 1) * WIN],
                        kh[:, jt * 128:(jt + 1) * 128],
                        qh,
                        start=True,
                        stop=True,
                    )
                e = heads.tile([128, 512], BF16, tag="es", name=f"es_{wp}_{wl}_{h}")
                nc.scalar.activation(
                    e[:], ps_sim[:], mybir.ActivationFunctionType.Exp, scale=SCALE
                )
                es_t[step] = e

            def emit_av(step):
                wl, h = step
                av = psV.tile([128, WIN], F32, tag="av", name=f"av_{wp}_{wl}_{h}")
                es = es_t.pop(step)
                for jt in range(2):
                    nc.tensor.matmul(
                        av[:],
                        v2b[wl][:, (jt * H + h) * 128:(jt * H + h + 1) * 128],
                        es[:, jt * WIN:(jt + 1) * WIN],
                        start=(jt == 0),
                        stop=(jt == 1),
                    )
                # reciprocal_approx_fast is broken for base-partition-64 APs,
                # so run it over all 128 partitions (same DVE cost: cost is
                # free-size-bound) and read only the S rows in the multiply.
                rs = heads.tile([128, WIN], F32, tag="rs", name=f"rs_{wp}_{wl}_{h}")
                nc.vector.reciprocal_approx_fast(rs[:], av[:])
                r0 = (h % 2) * 64
                nc.vector.tensor_mul(
                    o2T[r0:r0 + 64, (h // 2) * 512 + wl * WIN:
                        (h // 2) * 512 + (wl + 1) * WIN],
                    av[0:64, :],
                    rs[64:128, :],
                )

            emit_sim(steps[0])
            emit_sim(steps[1])
            for i in range(2, len(steps)):
                emit_sim(steps[i])
                emit_av(steps[i - 2])
            emit_av(steps[-2])
            emit_av(steps[-1])

            prev = o2T

        for g in range(4):
            emit_y_group(n_pair - 1, prev, g, 0)
            emit_y_group(n_pair - 1, prev, g, 1)


_CACHE = {}


def _build(n_win=N_WIN):
    key = n_win
    if key in _CACHE:
        return _CACHE[key]
    tok = n_win * WIN
    nc = bacc.Bacc(
        "TRN2", target_bir_lowering=False, debug=False, num_devices=N_CORES
    )
    xq = nc.dram_tensor("xq", [tok, D], BF16, kind="ExternalInput").ap()
    wq = nc.dram_tensor("Wq", [D, D], BF16, kind="ExternalInput").ap()
    wk = nc.dram_tensor("Wk", [D, D], BF16, kind="ExternalInput").ap()
    wv = nc.dram_tensor("Wv", [D, D], BF16, kind="ExternalInput").ap()
    wo = nc.dram_tensor("Wo", [D, D], BF16, kind="ExternalInput").ap()
    out = nc.dram_tensor("out", [tok, D], F32, kind="ExternalOutput").ap()
    with tile.TileContext(nc) as tc:
        _body(tc, xq, wq, wk, wv, wo, out, n_win)
    nc.compile()
    nc.m = get_hw_module(nc.m)
    _CACHE[key] = nc
    return nc


def run(query, Wq, Wk, Wv, Wo, bo, n_win=N_WIN, **spmd_kwargs):
    nc = _build(n_win)
    tok = n_win * WIN
    bf = ml_dtypes.bfloat16
    q2 = np.ascontiguousarray(
        np.asarray(query, dtype=np.float32).reshape(-1, D).astype(bf)
    )
    weights = {
        "Wq": np.ascontiguousarray(np.asarray(Wq, np.float32).astype(bf)),
        "Wk": np.ascontiguousarray(np.asarray(Wk, np.float32).astype(bf)),
        "Wv": np.ascontiguousarray(np.asarray(Wv, np.float32).astype(bf)),
        "Wo": np.ascontiguousarray(np.asarray(Wo, np.float32).astype(bf)),
    }
    in_maps = []
    for c in range(N_CORES):
        m = {"xq": q2[c * TOK:c * TOK + tok]}
        m.update(weights)
        in_maps.append(m)
    res = bass_utils.run_bass_kernel_spmd(
        nc, in_maps, core_ids=list(range(N_CORES)), **spmd_kwargs
    )
    outs = [res.results[c]["out"] for c in range(N_CORES)]
    return outs, res


def kernel(query, context, Wq, Wk, Wv, Wo, bo):
    outs, _ = run(query, Wq, Wk, Wv, Wo, bo)
    y = np.concatenate(outs, axis=0).reshape(B, N, D)
    bo = np.asarray(bo, np.float32)
    if bo.any():
        y = y + bo  # bias is structurally zero for this problem; host-add keeps exactness
    return y.astype(np.float32)


# revision 14
# speedup vs baseline: 1.4668x; 1.0061x over previous
"""Windowed local self-attention (CrossAttention module with the context-
overwrite bug faithfully reproduced) on 8 Trainium2 NeuronCores.

Full-input contract: kernel(**inputs) takes the unsharded tensors and
returns the full (4, 4096, 1024) output. Internally the 64 independent
windows of 256 tokens are data-parallel sharded 8-per-core; the four
projection weights are broadcast to every core. No collectives needed.

All matmul operands are bf16 (host-cast): 1 cycle/row on the PE, half
the SBUF/DMA traffic of fp32, and far less PE power draw than fp32 HIGH
mode (which triggered 50%-utilization periodic throttling in the fp32r
version). PSUM accumulation, softmax normalization and the final output
stay fp32.

Windows are processed in PAIRS (512 tokens) so every projection /
output matmul streams the maximum 512 moving rows per instruction,
hiding LDWEIGHTS under the row stream.

Per-core pipeline (window = 256 tokens, H=16 heads, DH=64):
  X  --PE transpose-->  XT [d, i]            (8 transposes per PSUM bank)
  qT = Wq.T @ X.T   (lhsT=Wq tiles,  rhs=XT)          [o, i]
  kT = Wk.T @ X.T                                      [o, i]
  v  = X @ Wv       (lhsT=XT tiles,  rhs=Wv)           [j, v|1]
       stored interleaved per head as [v_h (64) | ones (64)] so that
  per head h (per window):
    simT = kT_h.T-free @ qT_h   -> [j, i] in PSUM     (j on partitions)
    es   = exp(0.125 * simT)    (ACT, PSUM->SBUF, bf16)
    av   = [v_h|1].T-free @ es  -> [128, i] PSUM: rows 0-63 = o2u_h,
           rows 64-127 = column sums S_h (replicated) -- the softmax
           denominator comes free out of the AV matmul, no S matmul.
    rS   = 1/S                  (DVE reciprocal from PSUM rows 64:128)
    o2T  = o2u * rS             (DVE, bf16 [o, i] SBUF)
  Y = o2T.T @ Wo       (lhsT=o2T tiles, rhs=Wo; zero bias added host-side)
"""

import numpy as np
import ml_dtypes

import concourse.bass as bass
import concourse.mybir as mybir
import concourse.tile as tile
from concourse import bacc, bass_utils
from concourse.bass_interp import get_hw_module
from concourse.masks import make_identity

H = 16
DH = 64
WIN = 256
D = 1024
B = 4
N = 4096
N_CORES = 8
N_WIN_TOTAL = B * N // WIN          # 64
N_WIN = N_WIN_TOTAL // N_CORES      # 8 windows per core
TOK = N_WIN * WIN                   # 2048 token rows per core
PAIR = 2 * WIN                      # 512 tokens per window pair
SCALE = DH ** -0.5

F32 = mybir.dt.float32
BF16 = mybir.dt.bfloat16


def _body(tc, xq, wq, wk, wv, wo, out, n_win):
    nc = tc.nc
    from contextlib import ExitStack

    n_pair = n_win // 2

    with ExitStack() as ctx:
        singles = ctx.enter_context(tc.tile_pool(name="singles", bufs=1))
        xpool = ctx.enter_context(tc.tile_pool(name="xpool", bufs=2))
        acts = ctx.enter_context(tc.tile_pool(name="acts", bufs=1))
        heads = ctx.enter_context(tc.tile_pool(name="heads", bufs=3))
        ypool = ctx.enter_context(tc.tile_pool(name="ypool", bufs=2))
        psA = ctx.enter_context(tc.tile_pool(name="psA", bufs=3, space="PSUM"))
        psS = ctx.enter_context(tc.tile_pool(name="psS", bufs=3, space="PSUM"))
        psV = ctx.enter_context(tc.tile_pool(name="psV", bufs=2, space="PSUM"))

        # ---- constants / weights (resident all kernel) ----
        ident_f = singles.tile([128, 128], F32)
        make_identity(nc, ident_f[:])
        ident = singles.tile([128, 128], BF16)
        nc.vector.tensor_copy(ident[:], ident_f[:])

        # first pair's X before the big weight DMAs so transposes start early
        x_first = [xpool.tile([128, D], BF16, tag="x", bufs=8, name=f"x0_{i}")
                   for i in range(4)]
        for tt in range(4):
            nc.sync.dma_start(x_first[tt][:], xq[tt * 128:(tt + 1) * 128, :])

        wsb = {}
        for name, w in (("wq", wq), ("wk", wk), ("wv", wv), ("wo", wo)):
            t = singles.tile([128, 8 * D], BF16, tag=name, name=f"sb_{name}")
            for kt in range(8):
                nc.sync.dma_start(
                    t[:, kt * D:(kt + 1) * D], w[kt * 128:(kt + 1) * 128, :]
                )
            wsb[name] = t

        # v double-buffer: window w uses v2b[w % 2]; per-head layout
        # [v_h (64 cols) | ones (64 cols)] so AV' yields sums on rows 64+.
        v2b = []
        for i in range(2):
            t = singles.tile([128, 2 * H * 128], BF16, name=f"v2_{i}")
            ones_view = t[:].rearrange("p (j h c) -> p j h c", j=2, h=H)[:, :, :, DH:]
            nc.gpsimd.memset(ones_view, 1.0)
            v2b.append(t)

        def emit_tp_group(wp, x_sb, xt, g):
            # one PSUM bank: 8 transposes (dt = 2g, 2g+1) -> one DVE copy
            pt = psA.tile([128, 1024], BF16, tag="acc", name=f"pt_{wp}_{g}")
            for d2 in range(2):
                dt_ = 2 * g + d2
                for tt in range(4):
                    nc.tensor.transpose(
                        pt[:, d2 * 512 + tt * 128:d2 * 512 + (tt + 1) * 128],
                        x_sb[tt][:, dt_ * 128:(dt_ + 1) * 128],
                        ident[:],
                    )
            nc.vector.tensor_copy(xt[:, 2 * g * 512:(2 * g + 2) * 512], pt[:])

        def emit_y_group(wp, o2T, it, ec):
            row0 = wp * PAIR
            py = psA.tile([128, 512], F32, tag="acc", name=f"py_{wp}_{it}_{ec}")
            for kt2 in range(8):
                nc.tensor.matmul(
                    py[:],
                    o2T[:, kt2 * 512 + it * 128:kt2 * 512 + (it + 1) * 128],
                    wsb["wo"][:, kt2 * D + ec * 512:kt2 * D + (ec + 1) * 512],
                    start=(kt2 == 0),
                    stop=(kt2 == 7),
                )
            y_sb = ypool.tile([128, 512], F32, tag="y", name=f"y_{wp}_{it}_{ec}")
            nc.vector.tensor_copy(y_sb[:], py[:])
            nc.sync.dma_start(
                out[row0 + it * 128:row0 + (it + 1) * 128, ec * 512:(ec + 1) * 512],
                y_sb[:],
            )

        prev = None  # o2T of previous pair
        for wp in range(n_pair):
            row0 = wp * PAIR
            if wp == 0:
                x_sb = x_first
            else:
                x_sb = [xpool.tile([128, D], BF16, tag="x", bufs=8,
                                   name=f"x_{wp}_{i}")
                        for i in range(4)]
                for tt in range(4):
                    nc.sync.dma_start(
                        x_sb[tt][:], xq[row0 + tt * 128:row0 + (tt + 1) * 128, :]
                    )

            # transposes first (ready immediately), THEN prev pair's Y groups:
            # Y waits on prev attention's final DVE muls, and the in-order PE
            # queue would stall on it ahead of the independent transposes.
            xt = acts.tile([128, 8 * 512], BF16, tag="xt", bufs=2, name=f"xt_{wp}")
            for g in range(4):
                emit_tp_group(wp, x_sb, xt, g)
            if prev is not None:
                for g in range(4):
                    emit_y_group(wp - 1, prev, g, 0)
                    emit_y_group(wp - 1, prev, g, 1)

            # ---- qT, kT [128 o, 8*512 i] ----
            proj = {}
            for pname, wname in (("qT", "wq"), ("kT", "wk")):
                dst = acts.tile([128, 8 * 512], BF16, tag=pname, bufs=2,
                                name=f"{pname}_{wp}")
                wtile = wsb[wname]
                for ot in range(8):
                    pq = psA.tile([128, 512], F32, tag="acc",
                                  name=f"pq_{wp}_{pname}_{ot}")
                    for kt in range(8):
                        nc.tensor.matmul(
                            pq[:],
                            wtile[:, kt * D + ot * 128:kt * D + (ot + 1) * 128],
                            xt[:, kt * 512:(kt + 1) * 512],
                            start=(kt == 0),
                            stop=(kt == 7),
                        )
                    nc.vector.tensor_copy(dst[:, ot * 512:(ot + 1) * 512], pq[:])
                proj[pname] = dst
            qT, kT = proj["qT"], proj["kT"]

            # ---- v [128 j, (jt, h, v|1)] strided into v2b buffers ----
            for tt in range(4):
                wl, jt = tt // 2, tt % 2
                vdst = v2b[wl]
                for oc in range(2):
                    pv = psA.tile([128, 512], F32, tag="acc",
                                  name=f"pv_{wp}_{tt}_{oc}")
                    for kt in range(8):
                        nc.tensor.matmul(
                            pv[:],
                            xt[:, kt * 512 + tt * 128:kt * 512 + (tt + 1) * 128],
                            wsb["wv"][:, kt * D + oc * 512:kt * D + (oc + 1) * 512],
                            start=(kt == 0),
                            stop=(kt == 7),
                        )
                    dsl = vdst[:, jt * H * 128 + oc * 8 * 128:
                               jt * H * 128 + (oc + 1) * 8 * 128]
                    nc.scalar.copy(
                        dsl.rearrange("p (h c) -> p h c", h=8)[:, :, 0:DH], pv[:]
                    )

            # ---- attention: 32 (window, head) steps, software-pipelined ----
            o2T = acts.tile([128, 8 * 512], BF16, tag="o2T", bufs=2,
                            name=f"o2T_{wp}")
            steps = [(wl, h) for wl in range(2) for h in range(H)]
            es_t = {}

            def emit_sim(step):
                wl, h = step
                prow = (h % 2) * 64
                ocol = (h // 2) * 512 + wl * WIN
                qh = qT[prow:prow + 64, ocol:ocol + WIN]
                kh = kT[prow:prow + 64, ocol:ocol + WIN]
                ps_sim = psS.tile([128, 512], F32, tag="sim",
                                  name=f"sim_{wp}_{wl}_{h}")
                for jt in range(2):
                    nc.tensor.matmul(
                        ps_sim[:, jt * WIN:(jt + 1) * WIN],
                        kh[:, jt * 128:(jt + 1) * 128],
                        qh,
                        start=True,
                        stop=True,
                    )
                e = heads.tile([128, 512], BF16, tag="es", bufs=4, name=f"es_{wp}_{wl}_{h}")
                nc.scalar.activation(
                    e[:], ps_sim[:], mybir.ActivationFunctionType.Exp, scale=SCALE
                )
                es_t[step] = e

            def emit_av(step):
                wl, h = step
                av = psV.tile([128, WIN], F32, tag="av", name=f"av_{wp}_{wl}_{h}")
                es = es_t.pop(step)
                for jt in range(2):
                    nc.tensor.matmul(
                        av[:],
                        v2b[wl][:, (jt * H + h) * 128:(jt * H + h + 1) * 128],
                        es[:, jt * WIN:(jt + 1) * WIN],
                        start=(jt == 0),
                        stop=(jt == 1),
                    )
                # reciprocal_approx_fast is broken for base-partition-64 APs,
                # so run it over all 128 partitions (same DVE cost: cost is
                # free-size-bound) and read only the S rows in the multiply.
                rs = heads.tile([128, WIN], F32, tag="rs", name=f"rs_{wp}_{wl}_{h}")
                nc.vector.reciprocal_approx_fast(rs[:], av[:])
                r0 = (h % 2) * 64
                nc.vector.tensor_mul(
                    o2T[r0:r0 + 64, (h // 2) * 512 + wl * WIN:
                        (h // 2) * 512 + (wl + 1) * WIN],
                    av[0:64, :],
                    rs[64:128, :],
                )

            emit_sim(steps[0])
            emit_sim(steps[1])
            for i in range(2, len(steps)):
                emit_sim(steps[i])
                emit_av(steps[i - 2])
            emit_av(steps[-2])
            emit_av(steps[-1])

            prev = o2T

        for g in range(4):
            emit_y_group(n_pair - 1, prev, g, 0)
            emit_y_group(n_pair - 1, prev, g, 1)


_CACHE = {}


def _build(n_win=N_WIN):
    key = n_win
    if key in _CACHE:
        return _CACHE[key]
    tok = n_win * WIN
    nc = bacc.Bacc(
        "TRN2", target_bir_lowering=False, debug=False, num_devices=N_CORES
    )
    xq = nc.dram_tensor("xq", [tok, D], BF16, kind="ExternalInput").ap()
    wq = nc.dram_tensor("Wq", [D, D], BF16, kind="ExternalInput").ap()
    wk = nc.dram_tensor("Wk", [D, D], BF16, kind="ExternalInput").ap()
    wv = nc.dram_tensor("Wv", [D, D], BF16, kind="ExternalInput").ap()
    wo = nc.dram_tensor("Wo", [D, D], BF16, kind="ExternalInput").ap()
    out = nc.dram_tensor("out", [tok, D], F32, kind="ExternalOutput").ap()
    with tile.TileContext(nc) as tc:
        _body(tc, xq, wq, wk, wv, wo, out, n_win)
    nc.compile()
    nc.m = get_hw_module(nc.m)
    _CACHE[key] = nc
    return nc


def run(query, Wq, Wk, Wv, Wo, bo, n_win=N_WIN, **spmd_kwargs):
    nc = _build(n_win)
    tok = n_win * WIN
    bf = ml_dtypes.bfloat16
    q2 = np.ascontiguousarray(
        np.asarray(query, dtype=np.float32).reshape(-1, D).astype(bf)
    )
    weights = {
        "Wq": np.ascontiguousarray(np.asarray(Wq, np.float32).astype(bf)),
        "Wk": np.ascontiguousarray(np.asarray(Wk, np.float32).astype(bf)),
        "Wv": np.ascontiguousarray(np.asarray(Wv, np.float32).astype(bf)),
        "Wo": np.ascontiguousarray(np.asarray(Wo, np.float32).astype(bf)),
    }
    in_maps = []
    for c in range(N_CORES):
        m = {"xq": q2[c * TOK:c * TOK + tok]}
        m.update(weights)
        in_maps.append(m)
    res = bass_utils.run_bass_kernel_spmd(
        nc, in_maps, core_ids=list(range(N_CORES)), **spmd_kwargs
    )
    outs = [res.results[c]["out"] for c in range(N_CORES)]
    return outs, res


def kernel(query, context, Wq, Wk, Wv, Wo, bo):
    outs, _ = run(query, Wq, Wk, Wv, Wo, bo)
    y = np.concatenate(outs, axis=0).reshape(B, N, D)
    bo = np.asarray(bo, np.float32)
    if bo.any():
        y = y + bo  # bias is structurally zero for this problem; host-add keeps exactness
    return y.astype(np.float32)


# revision 16
# speedup vs baseline: 1.4958x; 1.0198x over previous
"""Windowed local self-attention (CrossAttention module with the context-
overwrite bug faithfully reproduced) on 8 Trainium2 NeuronCores.

Full-input contract: kernel(**inputs) takes the unsharded tensors and
returns the full (4, 4096, 1024) output. Internally the 64 independent
windows of 256 tokens are data-parallel sharded 8-per-core; the four
projection weights are broadcast to every core. No collectives needed.

All matmul operands are bf16 (host-cast): 1 cycle/row on the PE, half
the SBUF/DMA traffic of fp32, and far less PE power draw than fp32 HIGH
mode (which triggered 50%-utilization periodic throttling in the fp32r
version). PSUM accumulation, softmax normalization and the final output
stay fp32.

Windows are processed in PAIRS (512 tokens) so every projection /
output matmul streams the maximum 512 moving rows per instruction,
hiding LDWEIGHTS under the row stream.

Per-core pipeline (window = 256 tokens, H=16 heads, DH=64):
  X  --PE transpose-->  XT [d, i]            (8 transposes per PSUM bank)
  qT = Wq.T @ X.T   (lhsT=Wq tiles,  rhs=XT)          [o, i]
  kT = Wk.T @ X.T                                      [o, i]
  v  = X @ Wv       (lhsT=XT tiles,  rhs=Wv)           [j, v|1]
       stored interleaved per head as [v_h (64) | ones (64)] so that
  per head h (per window):
    simT = kT_h.T-free @ qT_h   -> [j, i] in PSUM     (j on partitions)
    es   = exp(0.125 * simT)    (ACT, PSUM->SBUF, bf16)
    av   = [v_h|1].T-free @ es  -> [128, i] PSUM: rows 0-63 = o2u_h,
           rows 64-127 = column sums S_h (replicated) -- the softmax
           denominator comes free out of the AV matmul, no S matmul.
    rS   = 1/S                  (DVE reciprocal from PSUM rows 64:128)
    o2T  = o2u * rS             (DVE, bf16 [o, i] SBUF)
  Y = o2T.T @ Wo       (lhsT=o2T tiles, rhs=Wo; zero bias added host-side)
"""

import numpy as np
import ml_dtypes

import concourse.bass as bass
import concourse.mybir as mybir
import concourse.tile as tile
from concourse import bacc, bass_utils
from concourse.bass_interp import get_hw_module
from concourse.masks import make_identity

H = 16
DH = 64
WIN = 256
D = 1024
B = 4
N = 4096
N_CORES = 8
N_WIN_TOTAL = B * N // WIN          # 64
N_WIN = N_WIN_TOTAL // N_CORES      # 8 windows per core
TOK = N_WIN * WIN                   # 2048 token rows per core
PAIR = 2 * WIN                      # 512 tokens per window pair
SCALE = DH ** -0.5

F32 = mybir.dt.float32
BF16 = mybir.dt.bfloat16


def _body(tc, xq, wq, wk, wv, wo, out, n_win):
    nc = tc.nc
    from contextlib import ExitStack

    n_pair = n_win // 2

    with ExitStack() as ctx:
        singles = ctx.enter_context(tc.tile_pool(name="singles", bufs=1))
        xpool = ctx.enter_context(tc.tile_pool(name="xpool", bufs=2))
        acts = ctx.enter_context(tc.tile_pool(name="acts", bufs=1))
        heads = ctx.enter_context(tc.tile_pool(name="heads", bufs=3))
        ypool = ctx.enter_context(tc.tile_pool(name="ypool", bufs=2))
        psA = ctx.enter_context(tc.tile_pool(name="psA", bufs=3, space="PSUM"))
        psS = ctx.enter_context(tc.tile_pool(name="psS", bufs=3, space="PSUM"))
        psV = ctx.enter_context(tc.tile_pool(name="psV", bufs=2, space="PSUM"))

        # ---- constants / weights (resident all kernel) ----
        ident_f = singles.tile([128, 128], F32)
        make_identity(nc, ident_f[:])
        ident = singles.tile([128, 128], BF16)
        nc.vector.tensor_copy(ident[:], ident_f[:])

        # first pair's X before the big weight DMAs so transposes start early
        x_first = [xpool.tile([128, D], BF16, tag="x", bufs=8, name=f"x0_{i}")
                   for i in range(4)]
        for tt in range(4):
            nc.sync.dma_start(x_first[tt][:], xq[tt * 128:(tt + 1) * 128, :])

        wsb = {}
        for name, w in (("wq", wq), ("wk", wk), ("wv", wv), ("wo", wo)):
            t = singles.tile([128, 8 * D], BF16, tag=name, name=f"sb_{name}")
            for kt in range(8):
                nc.sync.dma_start(
                    t[:, kt * D:(kt + 1) * D], w[kt * 128:(kt + 1) * 128, :]
                )
            wsb[name] = t

        # v double-buffer: window w uses v2b[w % 2]; per-head layout
        # [v_h (64 cols) | ones (64 cols)] so AV' yields sums on rows 64+.
        v2b = []
        for i in range(2):
            t = singles.tile([128, 2 * H * 128], BF16, name=f"v2_{i}")
            ones_view = t[:].rearrange("p (j h c) -> p j h c", j=2, h=H)[:, :, :, DH:]
            nc.gpsimd.memset(ones_view, 1.0)
            v2b.append(t)

        def emit_tp_group(wp, x_sb, xt, g):
            # one PSUM bank: 8 transposes (dt = 2g, 2g+1) -> one DVE copy
            pt = psA.tile([128, 1024], BF16, tag="acc", name=f"pt_{wp}_{g}")
            for d2 in range(2):
                dt_ = 2 * g + d2
                for tt in range(4):
                    nc.tensor.transpose(
                        pt[:, d2 * 512 + tt * 128:d2 * 512 + (tt + 1) * 128],
                        x_sb[tt][:, dt_ * 128:(dt_ + 1) * 128],
                        ident[:],
                    )
            nc.vector.tensor_copy(xt[:, 2 * g * 512:(2 * g + 2) * 512], pt[:])

        def emit_y_group(wp, o2T, it, ec):
            row0 = wp * PAIR
            py = psA.tile([128, 512], F32, tag="acc", name=f"py_{wp}_{it}_{ec}")
            for kt2 in range(8):
                nc.tensor.matmul(
                    py[:],
                    o2T[:, kt2 * 512 + it * 128:kt2 * 512 + (it + 1) * 128],
                    wsb["wo"][:, kt2 * D + ec * 512:kt2 * D + (ec + 1) * 512],
                    start=(kt2 == 0),
                    stop=(kt2 == 7),
                )
            y_sb = ypool.tile([128, 512], F32, tag="y", name=f"y_{wp}_{it}_{ec}")
            nc.vector.tensor_copy(y_sb[:], py[:])
            nc.sync.dma_start(
                out[row0 + it * 128:row0 + (it + 1) * 128, ec * 512:(ec + 1) * 512],
                y_sb[:],
            )

        prev = None  # o2T of previous pair
        for wp in range(n_pair):
            row0 = wp * PAIR
            if wp == 0:
                x_sb = x_first
            else:
                x_sb = [xpool.tile([128, D], BF16, tag="x", bufs=8,
                                   name=f"x_{wp}_{i}")
                        for i in range(4)]
                for tt in range(4):
                    nc.sync.dma_start(
                        x_sb[tt][:], xq[row0 + tt * 128:row0 + (tt + 1) * 128, :]
                    )

            # transposes first (ready immediately), THEN prev pair's Y groups:
            # Y waits on prev attention's final DVE muls, and the in-order PE
            # queue would stall on it ahead of the independent transposes.
            xt = acts.tile([128, 8 * 512], BF16, tag="xt", bufs=2, name=f"xt_{wp}")
            for g in range(4):
                emit_tp_group(wp, x_sb, xt, g)
            if prev is not None:
                for g in range(4):
                    emit_y_group(wp - 1, prev, g, 0)
                    emit_y_group(wp - 1, prev, g, 1)

            # ---- qT, kT [128 o, 8*512 i]; chains zipped q0,k0,q1,k1,... so
            # kT's low ot blocks are copied early (first sims need them) ----
            proj = {}
            for pname in ("qT", "kT"):
                proj[pname] = acts.tile([128, 8 * 512], BF16, tag=pname, bufs=2,
                                        name=f"{pname}_{wp}")
            for ot in range(8):
                for pname, wname in (("qT", "wq"), ("kT", "wk")):
                    wtile = wsb[wname]
                    pq = psA.tile([128, 512], F32, tag="acc",
                                  name=f"pq_{wp}_{pname}_{ot}")
                    for kt in range(8):
                        nc.tensor.matmul(
                            pq[:],
                            wtile[:, kt * D + ot * 128:kt * D + (ot + 1) * 128],
                            xt[:, kt * 512:(kt + 1) * 512],
                            start=(kt == 0),
                            stop=(kt == 7),
                        )
                    nc.vector.tensor_copy(
                        proj[pname][:, ot * 512:(ot + 1) * 512], pq[:]
                    )
            qT, kT = proj["qT"], proj["kT"]

            # ---- v [128 j, (jt, h, v|1)] strided into v2b buffers ----
            for tt in range(4):
                wl, jt = tt // 2, tt % 2
                vdst = v2b[wl]
                for oc in range(2):
                    pv = psA.tile([128, 512], F32, tag="acc",
                                  name=f"pv_{wp}_{tt}_{oc}")
                    for kt in range(8):
                        nc.tensor.matmul(
                            pv[:],
                            xt[:, kt * 512 + tt * 128:kt * 512 + (tt + 1) * 128],
                            wsb["wv"][:, kt * D + oc * 512:kt * D + (oc + 1) * 512],
                            start=(kt == 0),
                            stop=(kt == 7),
                        )
                    dsl = vdst[:, jt * H * 128 + oc * 8 * 128:
                               jt * H * 128 + (oc + 1) * 8 * 128]
                    nc.scalar.copy(
                        dsl.rearrange("p (h c) -> p h c", h=8)[:, :, 0:DH], pv[:]
                    )

            # ---- attention: 32 (window, head) steps, software-pipelined ----
            o2T = acts.tile([128, 8 * 512], BF16, tag="o2T", bufs=2,
                            name=f"o2T_{wp}")
            steps = [(wl, h) for wl in range(2) for h in range(H)]
            es_t = {}

            def emit_sim(step):
                wl, h = step
                prow = (h % 2) * 64
                ocol = (h // 2) * 512 + wl * WIN
                qh = qT[prow:prow + 64, ocol:ocol + WIN]
                kh = kT[prow:prow + 64, ocol:ocol + WIN]
                ps_sim = psS.tile([128, 512], F32, tag="sim",
                                  name=f"sim_{wp}_{wl}_{h}")
                for jt in range(2):
                    nc.tensor.matmul(
                        ps_sim[:, jt * WIN:(jt + 1) * WIN],
                        kh[:, jt * 128:(jt + 1) * 128],
                        qh,
                        start=True,
                        stop=True,
                    )
                e = heads.tile([128, 512], BF16, tag="es", bufs=4, name=f"es_{wp}_{wl}_{h}")
                nc.scalar.activation(
                    e[:], ps_sim[:], mybir.ActivationFunctionType.Exp, scale=SCALE
                )
                es_t[step] = e

            def emit_av(step):
                wl, h = step
                av = psV.tile([128, WIN], F32, tag="av", name=f"av_{wp}_{wl}_{h}")
                es = es_t.pop(step)
                for jt in range(2):
                    nc.tensor.matmul(
                        av[:],
                        v2b[wl][:, (jt * H + h) * 128:(jt * H + h + 1) * 128],
                        es[:, jt * WIN:(jt + 1) * WIN],
                        start=(jt == 0),
                        stop=(jt == 1),
                    )
                # reciprocal_approx_fast is broken for base-partition-64 APs,
                # so run it over all 128 partitions (same DVE cost: cost is
                # free-size-bound) and read only the S rows in the multiply.
                rs = heads.tile([128, WIN], F32, tag="rs", name=f"rs_{wp}_{wl}_{h}")
                nc.vector.reciprocal_approx_fast(rs[:], av[:])
                r0 = (h % 2) * 64
                nc.vector.tensor_mul(
                    o2T[r0:r0 + 64, (h // 2) * 512 + wl * WIN:
                        (h // 2) * 512 + (wl + 1) * WIN],
                    av[0:64, :],
                    rs[64:128, :],
                )

            last = wp == n_pair - 1
            for i in range(3):
                emit_sim(steps[i])
            for i in range(3, len(steps)):
                emit_sim(steps[i])
                emit_av(steps[i - 3])
                if last and i - 3 == 15:
                    # window 0 fully normalized: start its Y groups now so the
                    # tail overlaps with window 1's attention
                    for g in range(2):
                        emit_y_group(wp, o2T, g, 0)
                        emit_y_group(wp, o2T, g, 1)
            for i in range(len(steps) - 3, len(steps)):
                emit_av(steps[i])

            prev = o2T

        for g in range(2, 4):
            emit_y_group(n_pair - 1, prev, g, 0)
            emit_y_group(n_pair - 1, prev, g, 1)


_CACHE = {}


def _build(n_win=N_WIN):
    key = n_win
    if key in _CACHE:
        return _CACHE[key]
    tok = n_win * WIN
    nc = bacc.Bacc(
        "TRN2", target_bir_lowering=False, debug=False, num_devices=N_CORES
    )
    xq = nc.dram_tensor("xq", [tok, D], BF16, kind="ExternalInput").ap()
    wq = nc.dram_tensor("Wq", [D, D], BF16, kind="ExternalInput").ap()
    wk = nc.dram_tensor("Wk", [D, D], BF16, kind="ExternalInput").ap()
    wv = nc.dram_tensor("Wv", [D, D], BF16, kind="ExternalInput").ap()
    wo = nc.dram_tensor("Wo", [D, D], BF16, kind="ExternalInput").ap()
    out = nc.dram_tensor("out", [tok, D], F32, kind="ExternalOutput").ap()
    with tile.TileContext(nc) as tc:
        _body(tc, xq, wq, wk, wv, wo, out, n_win)
    nc.compile()
    nc.m = get_hw_module(nc.m)
    _CACHE[key] = nc
    return nc


def run(query, Wq, Wk, Wv, Wo, bo, n_win=N_WIN, **spmd_kwargs):
    nc = _build(n_win)
    tok = n_win * WIN
    bf = ml_dtypes.bfloat16
    q2 = np.ascontiguousarray(
        np.asarray(query, dtype=np.float32).reshape(-1, D).astype(bf)
    )
    weights = {
        "Wq": np.ascontiguousarray(np.asarray(Wq, np.float32).astype(bf)),
        "Wk": np.ascontiguousarray(np.asarray(Wk, np.float32).astype(bf)),
        "Wv": np.ascontiguousarray(np.asarray(Wv, np.float32).astype(bf)),
        "Wo": np.ascontiguousarray(np.asarray(Wo, np.float32).astype(bf)),
    }
    in_maps = []
    for c in range(N_CORES):
        m = {"xq": q2[c * TOK:c * TOK + tok]}
        m.update(weights)
        in_maps.append(m)
    res = bass_utils.run_bass_kernel_spmd(
        nc, in_maps, core_ids=list(range(N_CORES)), **spmd_kwargs
    )
    outs = [res.results[c]["out"] for c in range(N_CORES)]
    return outs, res


def kernel(query, context, Wq, Wk, Wv, Wo, bo):
    outs, _ = run(query, Wq, Wk, Wv, Wo, bo)
    y = np.concatenate(outs, axis=0).reshape(B, N, D)
    bo = np.asarray(bo, np.float32)
    if bo.any():
        y = y + bo  # bias is structurally zero for this problem; host-add keeps exactness
    return y.astype(np.float32)


# revision 17
# speedup vs baseline: 1.5902x; 1.0632x over previous
"""Windowed local self-attention (CrossAttention module with the context-
overwrite bug faithfully reproduced) on 8 Trainium2 NeuronCores.

Full-input contract: kernel(**inputs) takes the unsharded tensors and
returns the full (4, 4096, 1024) output. Internally the 64 independent
windows of 256 tokens are data-parallel sharded 8-per-core; the four
projection weights are broadcast to every core. No collectives needed.

All matmul operands are bf16 (host-cast): 1 cycle/row on the PE, half
the SBUF/DMA traffic of fp32, far less PE power than fp32 HIGH mode
(which triggered 50% periodic throttling in the fp32r version). PSUM
accumulation, softmax normalization and the final output stay fp32.

Key structure:
- X is transposed on the HOST: the kernel DMAs X^T tiles straight into
  SBUF, so no PE transposes / identity preamble at all.
- Windows processed in PAIRS (512 tokens): every projection/output
  matmul streams the max 512 moving rows, hiding LDWEIGHTS.
- V is stored interleaved per head as [v_h (64) | ones (64)]; the AV
  matmul then emits the attention numerator on rows 0-63 AND the
  softmax denominator (replicated) on rows 64-127 -- no row-sum matmul.
- The AV results of a head pair share one PSUM bank -> one reciprocal
  per two heads.
- Software pipelining: the attention phase of pair p is DVE/ACT-paced,
  so the projection chains of pair p+1 (and pair p's output-projection
  chains) are interleaved into its step loop to keep the PE streaming.

Per-core pipeline (window = 256 tokens, H=16 heads, DH=64):
  qT = Wq.T @ X.T   (lhsT=Wq tiles,  rhs=XT)          [o, i]
  kT = Wk.T @ X.T                                      [o, i]
  v  = X @ Wv       (lhsT=XT tiles,  rhs=Wv)           [j, v|1]
  per (window, head):
    simT = kT_h.T-free @ qT_h   -> [j, i] in PSUM     (j on partitions)
    es   = exp(0.125 * simT)    (ACT, PSUM->SBUF, bf16)
    av   = [v_h|1].T-free @ es  -> [128, i] PSUM
    rS   = 1/S   (one DVE reciprocal per head pair, full PSUM bank)
    o2T  = o2u * rS             (DVE, bf16 [o, i] SBUF)
  Y = o2T.T @ Wo       (lhsT=o2T tiles, rhs=Wo; zero bias added host-side)
"""

import numpy as np
import ml_dtypes

import concourse.bass as bass
import concourse.mybir as mybir
import concourse.tile as tile
from concourse import bacc, bass_utils
from concourse.bass_interp import get_hw_module

H = 16
DH = 64
WIN = 256
D = 1024
B = 4
N = 4096
N_CORES = 8
N_WIN_TOTAL = B * N // WIN          # 64
N_WIN = N_WIN_TOTAL // N_CORES      # 8 windows per core
TOK = N_WIN * WIN                   # 2048 token rows per core
PAIR = 2 * WIN                      # 512 tokens per window pair
SCALE = DH ** -0.5

F32 = mybir.dt.float32
BF16 = mybir.dt.bfloat16


def _body(tc, xqT, wq, wk, wv, wo, out, n_win):
    nc = tc.nc
    from contextlib import ExitStack

    n_pair = n_win // 2

    with ExitStack() as ctx:
        singles = ctx.enter_context(tc.tile_pool(name="singles", bufs=1))
        acts = ctx.enter_context(tc.tile_pool(name="acts", bufs=1))
        heads = ctx.enter_context(tc.tile_pool(name="heads", bufs=3))
        ypool = ctx.enter_context(tc.tile_pool(name="ypool", bufs=2))
        psA = ctx.enter_context(tc.tile_pool(name="psA", bufs=3, space="PSUM"))
        psS = ctx.enter_context(tc.tile_pool(name="psS", bufs=3, space="PSUM"))
        psV = ctx.enter_context(tc.tile_pool(name="psV", bufs=2, space="PSUM"))

        def emit_xt_dma(wp):
            t = acts.tile([128, 8 * 512], BF16, tag="xt", bufs=2,
                          name=f"xt_{wp}")
            for dt in range(8):
                nc.sync.dma_start(
                    t[:, dt * 512:(dt + 1) * 512],
                    xqT[dt * 128:(dt + 1) * 128, wp * PAIR:(wp + 1) * PAIR],
                )
            return t

        # first pair's XT before the big weight DMAs so chains start early
        xt0 = emit_xt_dma(0)

        wsb = {}
        for name, w in (("wq", wq), ("wk", wk), ("wv", wv), ("wo", wo)):
            t = singles.tile([128, 8 * D], BF16, tag=name, name=f"sb_{name}")
            for kt in range(8):
                nc.sync.dma_start(
                    t[:, kt * D:(kt + 1) * D], w[kt * 128:(kt + 1) * 128, :]
                )
            wsb[name] = t

        # v buffers: pair parity x window -> 4 buffers; per-head layout
        # [v_h (64 cols) | ones (64 cols)] so AV' yields sums on rows 64+.
        v2b = []
        for i in range(4):
            t = singles.tile([128, 2 * H * 128], BF16, name=f"v2_{i}")
            ones_view = t[:].rearrange("p (j h c) -> p j h c", j=2, h=H)[:, :, :, DH:]
            nc.gpsimd.memset(ones_view, 1.0)
            v2b.append(t)

        def proj_chains(wp, xt):
            """qT/kT/v chains for pair wp as a list of zero-arg closures."""
            proj = {}
            for pname in ("qT", "kT"):
                proj[pname] = acts.tile([128, 8 * 512], BF16, tag=pname,
                                        bufs=2, name=f"{pname}_{wp}")
            chains = []
            for ot in range(8):
                for pname, wname in (("qT", "wq"), ("kT", "wk")):
                    def qk_chain(ot=ot, pname=pname, wname=wname):
                        pq = psA.tile([128, 512], F32, tag="acc",
                                      name=f"pq_{wp}_{pname}_{ot}")
                        wtile = wsb[wname]
                        for kt in range(8):
                            nc.tensor.matmul(
                                pq[:],
                                wtile[:, kt * D + ot * 128:kt * D + (ot + 1) * 128],
                                xt[:, kt * 512:(kt + 1) * 512],
                                start=(kt == 0),
                                stop=(kt == 7),
                            )
                        nc.vector.tensor_copy(
                            proj[pname][:, ot * 512:(ot + 1) * 512], pq[:]
                        )
                    chains.append(qk_chain)
            for tt in range(4):
                for oc in range(2):
                    def v_chain(tt=tt, oc=oc):
                        wl, jt = tt // 2, tt % 2
                        pv = psA.tile([128, 512], F32, tag="acc",
                                      name=f"pv_{wp}_{tt}_{oc}")
                        for kt in range(8):
                            nc.tensor.matmul(
                                pv[:],
                                xt[:, kt * 512 + tt * 128:kt * 512 + (tt + 1) * 128],
                                wsb["wv"][:, kt * D + oc * 512:kt * D + (oc + 1) * 512],
                                start=(kt == 0),
                                stop=(kt == 7),
                            )
                        vdst = v2b[(wp % 2) * 2 + wl]
                        dsl = vdst[:, jt * H * 128 + oc * 8 * 128:
                                   jt * H * 128 + (oc + 1) * 8 * 128]
                        nc.scalar.copy(
                            dsl.rearrange("p (h c) -> p h c", h=8)[:, :, 0:DH],
                            pv[:],
                        )
                    chains.append(v_chain)
            return proj, chains

        def emit_y_group(wp, o2T, it, ec):
            row0 = wp * PAIR
            py = psA.tile([128, 512], F32, tag="acc", name=f"py_{wp}_{it}_{ec}")
            for kt2 in range(8):
                nc.tensor.matmul(
                    py[:],
                    o2T[:, kt2 * 512 + it * 128:kt2 * 512 + (it + 1) * 128],
                    wsb["wo"][:, kt2 * D + ec * 512:kt2 * D + (ec + 1) * 512],
                    start=(kt2 == 0),
                    stop=(kt2 == 7),
                )
            y_sb = ypool.tile([128, 512], F32, tag="y", name=f"y_{wp}_{it}_{ec}")
            nc.vector.tensor_copy(y_sb[:], py[:])
            nc.sync.dma_start(
                out[row0 + it * 128:row0 + (it + 1) * 128, ec * 512:(ec + 1) * 512],
                y_sb[:],
            )

        def attention(wp, qT, kT, o2T, extra):
            """32 (window, head) steps; `extra` chains are paced through the
            step loop to keep the PE streaming while DVE/ACT normalize."""
            steps = [(wl, h) for wl in range(2) for h in range(H)]
            es_t = {}
            av_t = {}

            def emit_sim(i):
                wl, h = steps[i]
                prow = (h % 2) * 64
                ocol = (h // 2) * 512 + wl * WIN
                qh = qT[prow:prow + 64, ocol:ocol + WIN]
                kh = kT[prow:prow + 64, ocol:ocol + WIN]
                ps_sim = psS.tile([128, 512], F32, tag="sim",
                                  name=f"sim_{wp}_{wl}_{h}")
                for jt in range(2):
                    nc.tensor.matmul(
                        ps_sim[:, jt * WIN:(jt + 1) * WIN],
                        kh[:, jt * 128:(jt + 1) * 128],
                        qh,
                        start=True,
                        stop=True,
                    )
                e = heads.tile([128, 512], BF16, tag="es", bufs=4,
                               name=f"es_{wp}_{wl}_{h}")
                nc.scalar.activation(
                    e[:], ps_sim[:], mybir.ActivationFunctionType.Exp, scale=SCALE
                )
                es_t[i] = e

            def emit_av(i):
                wl, h = steps[i]
                if h % 2 == 0:
                    av_t[i // 2] = psV.tile([128, 512], F32, tag="av",
                                            name=f"av_{wp}_{wl}_{h}")
                av2 = av_t[i // 2]
                c0 = (h % 2) * WIN
                es = es_t.pop(i)
                for jt in range(2):
                    nc.tensor.matmul(
                        av2[:, c0:c0 + WIN],
                        v2b[(wp % 2) * 2 + wl][:, (jt * H + h) * 128:
                                               (jt * H + h + 1) * 128],
                        es[:, jt * WIN:(jt + 1) * WIN],
                        start=(jt == 0),
                        stop=(jt == 1),
                    )

            def emit_epilogue(p):
                # heads 2p', 2p'+1 of window wl share av bank p: one recip
                av2 = av_t.pop(p)
                wl, h0 = steps[2 * p]
                rs = heads.tile([128, 512], F32, tag="rs", name=f"rs_{wp}_{p}")
                nc.vector.reciprocal_approx_fast(rs[:], av2[:])
                for hh in range(2):
                    h = h0 + hh
                    c0 = hh * WIN
                    r0 = (h % 2) * 64
                    nc.vector.tensor_mul(
                        o2T[r0:r0 + 64, (h // 2) * 512 + wl * WIN:
                            (h // 2) * 512 + (wl + 1) * WIN],
                        av2[0:64, c0:c0 + WIN],
                        rs[64:128, c0:c0 + WIN],
                    )

            n_extra = len(extra)
            ch_i = 0
            for i in range(3):
                emit_sim(i)
            for i in range(len(steps)):
                if i + 3 < len(steps):
                    emit_sim(i + 3)
                emit_av(i)
                if i % 2 == 1:
                    emit_epilogue(i // 2)
                if i == 15:
                    # window 0 fully normalized: its Y groups can flow now
                    for g in range(2):
                        emit_y_group(wp, o2T, g, 0)
                        emit_y_group(wp, o2T, g, 1)
                while ch_i * len(steps) < n_extra * (i + 1):
                    extra[ch_i]()
                    ch_i += 1
            for g in range(2, 4):
                emit_y_group(wp, o2T, g, 0)
                emit_y_group(wp, o2T, g, 1)

        proj, chains0 = proj_chains(0, xt0)
        for ch in chains0:
            ch()
        for wp in range(n_pair):
            o2T = acts.tile([128, 8 * 512], BF16, tag="o2T", bufs=2,
                            name=f"o2T_{wp}")
            if wp + 1 < n_pair:
                xt_next = emit_xt_dma(wp + 1)
                proj_next, chains_next = proj_chains(wp + 1, xt_next)
            else:
                proj_next, chains_next = None, []
            attention(wp, proj["qT"], proj["kT"], o2T, chains_next)
            proj = proj_next


_CACHE = {}


def _build(n_win=N_WIN):
    key = n_win
    if key in _CACHE:
        return _CACHE[key]
    tok = n_win * WIN
    nc = bacc.Bacc(
        "TRN2", target_bir_lowering=False, debug=False, num_devices=N_CORES
    )
    xqT = nc.dram_tensor("xqT", [D, tok], BF16, kind="ExternalInput").ap()
    wq = nc.dram_tensor("Wq", [D, D], BF16, kind="ExternalInput").ap()
    wk = nc.dram_tensor("Wk", [D, D], BF16, kind="ExternalInput").ap()
    wv = nc.dram_tensor("Wv", [D, D], BF16, kind="ExternalInput").ap()
    wo = nc.dram_tensor("Wo", [D, D], BF16, kind="ExternalInput").ap()
    out = nc.dram_tensor("out", [tok, D], F32, kind="ExternalOutput").ap()
    with tile.TileContext(nc) as tc:
        _body(tc, xqT, wq, wk, wv, wo, out, n_win)
    nc.compile()
    nc.m = get_hw_module(nc.m)
    _CACHE[key] = nc
    return nc


def run(query, Wq, Wk, Wv, Wo, bo, n_win=N_WIN, **spmd_kwargs):
    nc = _build(n_win)
    tok = n_win * WIN
    bf = ml_dtypes.bfloat16
    q2 = np.asarray(query, dtype=np.float32).reshape(-1, D).astype(bf)
    weights = {
        "Wq": np.ascontiguousarray(np.asarray(Wq, np.float32).astype(bf)),
        "Wk": np.ascontiguousarray(np.asarray(Wk, np.float32).astype(bf)),
        "Wv": np.ascontiguousarray(np.asarray(Wv, np.float32).astype(bf)),
        "Wo": np.ascontiguousarray(np.asarray(Wo, np.float32).astype(bf)),
    }
    in_maps = []
    for c in range(N_CORES):
        m = {"xqT": np.ascontiguousarray(q2[c * TOK:c * TOK + tok].T)}
        m.update(weights)
        in_maps.append(m)
    res = bass_utils.run_bass_kernel_spmd(
        nc, in_maps, core_ids=list(range(N_CORES)), **spmd_kwargs
    )
    outs = [res.results[c]["out"] for c in range(N_CORES)]
    return outs, res


def kernel(query, context, Wq, Wk, Wv, Wo, bo):
    outs, _ = run(query, Wq, Wk, Wv, Wo, bo)
    y = np.concatenate(outs, axis=0).reshape(B, N, D)
    bo = np.asarray(bo, np.float32)
    if bo.any():
        y = y + bo  # bias is structurally zero for this problem; host-add keeps exactness
    return y.astype(np.float32)
